# revision 1
# baseline (speedup 1.0000x reference)
"""Trainium2 Bass kernel for nn_Backbone1_62947040690721.

Data-parallel over the fused B*NV block axis: 336 independent per-series
problems, 42 per NeuronCore across 8 cores. All weights replicated.

Layouts (per core, 42 blocks):
  - row layout: (block,patch) rows on partitions (128 rows = 2 blocks/tile)
  - T layout:   features on partitions, l = concat of blocks on the free axis,
                padded to 67 cols/block (3 zero "gap" cols + 64 data cols).
                The gaps give causal-conv zero padding and selective-scan
                state resets between blocks for free.
The selective scan runs as hardware tensor_tensor_scan instructions, one per
(state dim d, channel chunk), scanning 7 blocks' timelines per call; scan
chunks are block-aligned so every chunk self-resets at its leading gap cols.
"""

import sys

sys.path.insert(0, "/opt/trn_rl_repo")

import numpy as np

import concourse.bass as bass
import concourse.mybir as mybir
import concourse.tile as tile
from concourse import bacc
from concourse.bass_utils import run_bass_kernel_spmd

F32 = mybir.dt.float32
F32R = mybir.dt.float32r
AF = mybir.ActivationFunctionType
OP = mybir.AluOpType
AX = mybir.AxisListType

# model dims
B, T, NV = 16, 512, 21
PS, STRIDE, PRED = 16, 8, 96
DM, DS, DC = 128, 16, 4
DIN = 2 * DM          # 256
DTR = 8
S_EA = 512
PN = (T - PS) // STRIDE + 1 + 1  # 64
EPS = 1e-5

NCORES = 8
NBLK = B * NV          # 336
RPC = NBLK // NCORES   # 42 blocks per core
NROW = RPC * PN        # 2688 compact rows per core
NRT = NROW // 128      # 21 row tiles
GAP = 3                # zero-pad cols before each block
LP = PN + GAP          # 67 padded cols per block
LT = RPC * LP          # 2814 padded timeline length
SCB = 7                # blocks per scan chunk
SCW = SCB * LP         # 469 scan chunk width (>=256 keeps fp32r full rate)
NSC = RPC // SCB       # 6 scan chunks
POISON = 1.0e30

DEBUG = False          # set True (before first kernel build) for stage taps
REPEAT = 1             # build the body N times (for differential timing)

_cache = {}


def _r(x):
    return np.ascontiguousarray(np.asarray(x, dtype=np.float32))


def prep_inputs(inputs):
    """Full inputs -> per-core input maps (pure data movement on host)."""
    x = _r(inputs["x"])
    xbn = np.ascontiguousarray(x.transpose(0, 2, 1).reshape(NBLK, T))
    xp = np.concatenate([xbn, np.repeat(xbn[:, -1:], STRIDE, axis=1)], axis=1)
    idx = np.arange(PN)[:, None] * STRIDE + np.arange(PS)[None, :]
    pat = xp[:, idx]                                     # (336, 64, 16)
    patT = np.ascontiguousarray(pat.transpose(2, 0, 1))  # (16, 336, 64)
    wv = np.tile(_r(inputs["revin_w"]), B).reshape(NBLK, 1)
    bv = np.tile(_r(inputs["revin_b"]), B).reshape(NBLK, 1)

    import ml_dtypes
    mlp2_wT = _r(inputs["mlp2_w"]).T       # (8192, 192)
    w2s = np.ascontiguousarray(
        mlp2_wT.reshape(PN, DM, 2 * PRED).transpose(1, 0, 2)
    ).astype(ml_dtypes.bfloat16)  # (128, 64, 192) bf16

    shared = {
        "mlp1_wT": _r(inputs["mlp1_w"]).T.copy(),          # (16,128)
        "mlp1_b_row": _r(inputs["mlp1_b"]).reshape(1, DM),
        "mk_wT": _r(inputs["mk_w"]).T.copy(),              # (128,512)
        "mv_wT": _r(inputs["mv_w"]).T.copy(),              # (512,128)
        "ln_w_row": _r(inputs["ln_w"]).reshape(1, DM),
        "ln_b_row": _r(inputs["ln_b"]).reshape(1, DM),
        "in_proj_wT": _r(inputs["in_proj_w"]).T.copy(),    # (128,512)
        "conv_w2": _r(inputs["conv_w"])[:, 0, :].copy(),   # (256,4)
        "conv_b_col": _r(inputs["conv_b"]).reshape(DIN, 1),
        "x_proj_wT": _r(inputs["x_proj_w"]).T.copy(),      # (256,40)
        "dt_proj_wT": _r(inputs["dt_proj_w"]).T.copy(),    # (8,256)
        "dt_proj_b_col": _r(inputs["dt_proj_b"]).reshape(DIN, 1),
        "A_log_in": _r(inputs["A_log"]),                   # (256,16)
        "D_col": _r(inputs["D_ssm"]).reshape(DIN, 1),
        "out_proj_wT": _r(inputs["out_proj_w"]).T.copy(),  # (256,128)
        "w2s": w2s,                                        # (128,64,192)
        "mlp2_b_col": _r(inputs["mlp2_b"]).reshape(2 * PRED, 1),
        "mlp3_wT": _r(inputs["mlp3_w"]).T.copy(),          # (192,96)
        "mlp3_b_row": _r(inputs["mlp3_b"]).reshape(1, PRED),
    }
    in_maps = []
    for c in range(NCORES):
        lo, hi = c * RPC, (c + 1) * RPC
        m = dict(shared)
        m["xrow"] = np.ascontiguousarray(xbn[lo:hi])                  # (42,512)
        m["patT"] = np.ascontiguousarray(patT[:, lo:hi, :]).reshape(PS, NROW)
        m["wv"] = np.ascontiguousarray(wv[lo:hi])
        m["bv"] = np.ascontiguousarray(bv[lo:hi])
        in_maps.append(m)
    return in_maps


def assemble(results):
    outs = np.concatenate([r["out"] for r in results], axis=0)  # (336, 96)
    out = outs.reshape(B, NV, PRED).transpose(0, 2, 1)
    return np.ascontiguousarray(out.astype(np.float32))


# ---------------------------------------------------------------------------
# program builder
# ---------------------------------------------------------------------------

def _decl_inputs(nc):
    d = {}
    spec = {
        "xrow": (RPC, T), "patT": (PS, NROW), "wv": (RPC, 1), "bv": (RPC, 1),
        "mlp1_wT": (PS, DM), "mlp1_b_row": (1, DM),
        "mk_wT": (DM, S_EA), "mv_wT": (S_EA, DM),
        "ln_w_row": (1, DM), "ln_b_row": (1, DM),
        "in_proj_wT": (DM, 2 * DIN),
        "conv_w2": (DIN, DC), "conv_b_col": (DIN, 1),
        "x_proj_wT": (DIN, DTR + 2 * DS),
        "dt_proj_wT": (DTR, DIN), "dt_proj_b_col": (DIN, 1),
        "A_log_in": (DIN, DS), "D_col": (DIN, 1),
        "out_proj_wT": (DIN, DM),
        "w2s": (DM, PN, 2 * PRED), "mlp2_b_col": (2 * PRED, 1),
        "mlp3_wT": (2 * PRED, PRED), "mlp3_b_row": (1, PRED),
    }
    for name, shape in spec.items():
        dty = mybir.dt.bfloat16 if name == "w2s" else F32
        d[name] = nc.dram_tensor(name, list(shape), dty,
                                 kind="ExternalInput").ap()
    return d


def build_program():
    key = ("nc", REPEAT, DEBUG)
    if key in _cache:
        return _cache[key]
    nc = bacc.Bacc("TRN2", target_bir_lowering=False, debug=False,
                   num_devices=NCORES)
    IN = _decl_inputs(nc)
    out_d = nc.dram_tensor("out", [RPC, PRED], F32, kind="ExternalOutput").ap()

    dbg = {}
    if DEBUG:
        for name, shape in [
            ("d_hT", (DM, NROW)), ("d_hbT", (DM, NROW)),
            ("d_xc2T", (DIN, LT)), ("d_deltaT", (DIN, LT)),
            ("d_duT", (DIN, LT)), ("d_y2T", (DIN, NROW)),
            ("d_moT", (DM, NROW)), ("d_dblT", (DTR + 2 * DS, LT)),
            ("d_dblB", (DS, LT)), ("d_dblC", (DS, LT)),
            ("d_stats", (RPC, 6)),
        ]:
            dty = (mybir.dt.bfloat16
                   if name in ("d_dblB", "d_dblC", "d_moT") else F32)
            dbg[name] = nc.dram_tensor(name, list(shape), dty,
                                       kind="ExternalOutput").ap()

    from contextlib import ExitStack
    from concourse.masks import make_identity
    from concourse.tile import add_dep_helper

    with tile.TileContext(nc) as tc:
      for _rep in range(REPEAT):
       with ExitStack() as ctx:
        P = lambda **kw: ctx.enter_context(tc.tile_pool(**kw))
        wpool = P(name="weights", bufs=1)
        cpool = P(name="consts", bufs=1)
        spool = P(name="statp", bufs=1)
        big = P(name="bigact", bufs=1)
        work = P(name="work", bufs=2)
        work2 = P(name="work2", bufs=2)
        scanp_cm = tc.tile_pool(name="scanp", bufs=3)
        scanp = scanp_cm.__enter__()
        # PSUM: mm(2 banks) + bc(4 banks) + y(2 banks) = 8 banks
        ps_mm = P(name="ps_mm", bufs=2, space="PSUM")
        ps_bc = P(name="ps_bc", bufs=2, space="PSUM")
        ps_y = P(name="ps_y", bufs=1, space="PSUM")

        dt = F32

        def dma(dst, src):
            nc.sync.dma_start(out=dst, in_=src)

        def mm_tile(shape, tag="mm"):
            return ps_mm.tile(list(shape), dt, tag=tag, name=tag)

        # ---- constants / weights to SBUF ----
        ident = cpool.tile([128, 128], dt)
        make_identity(nc, ident[:])
        identb = cpool.tile([128, 128], mybir.dt.bfloat16)
        make_identity(nc, identb[:])
        ones1 = cpool.tile([1, 128], dt)
        nc.vector.memset(ones1[:], 1.0)
        ones16 = cpool.tile([PS, 1], dt)
        nc.vector.memset(ones16[:], 1.0)
        epsc = cpool.tile([128, 1], dt)
        nc.vector.memset(epsc[:], EPS)

        # input data first so stage A starts immediately
        xr = big.tile([RPC, T], dt, tag="xrow")
        dma(xr[:], IN["xrow"])
        wv = spool.tile([RPC, 1], dt)
        dma(wv[:], IN["wv"])
        bv = spool.tile([RPC, 1], dt)
        dma(bv[:], IN["bv"])

        w = {}
        for name, shape in [
            ("mlp1_wT", (PS, DM)), ("mk_wT", (DM, S_EA)),
            ("in_proj_wT", (DM, 2 * DIN)), ("dt_proj_wT", (DTR, DIN)),
        ]:
            tl = wpool.tile(list(shape), dt, tag=name)
            dma(tl[:], IN[name])
            w[name] = tl
        # channel-chunked weights (DIN=256 or 192 rows -> per-128 tiles)
        for name, shape in [
            ("conv_w2", (DIN, DC)), ("conv_b_col", (DIN, 1)),
            ("x_proj_wT", (DIN, DTR + 2 * DS)), ("dt_proj_b_col", (DIN, 1)),
            ("D_col", (DIN, 1)), ("out_proj_wT", (DIN, DM)),
            ("mlp2_b_col", (2 * PRED, 1)), ("mlp3_wT", (2 * PRED, PRED)),
        ]:
            rows = shape[0]
            parts = []
            for cc in range((rows + 127) // 128):
                r0 = cc * 128
                r1 = min(rows, r0 + 128)
                tl = wpool.tile([r1 - r0, shape[1]], dt, tag=f"{name}{cc}")
                dma(tl[:], IN[name][r0:r1, :])
                parts.append(tl)
            w[name] = parts

        # mv_aug: mv_wT chunks with an appended ones column (for softmax sums)
        mv_aug = wpool.tile([128, 4 * (DM + 1)], dt)
        for sc in range(4):
            dma(mv_aug[:, sc * 129:sc * 129 + DM],
                IN["mv_wT"][sc * 128:(sc + 1) * 128, :])
            nc.vector.memset(mv_aug[:, sc * 129 + DM:(sc + 1) * 129], 1.0)

        # A = -exp(A_log), (128,16) per channel chunk
        A_sb = []
        for cc in range(2):
            raw = work.tile([128, DS], dt, tag="araw")
            dma(raw[:], IN["A_log_in"][cc * 128:(cc + 1) * 128, :])
            ex = work.tile([128, DS], dt, tag="aexp")
            nc.scalar.activation(ex[:], raw[:], AF.Exp)
            neg = wpool.tile([128, DS], dt, tag=f"A_{cc}")
            nc.vector.tensor_scalar_mul(neg[:], ex[:], -1.0)
            A_sb.append(neg)

        # broadcast a (1,width) DRAM row -> (128,width) SBUF tile
        def bcast_row(dram_row, width, tag):
            row = work.tile([1, width], dt, tag="brow")
            dma(row[:], dram_row)
            ps = mm_tile([128, width])
            nc.tensor.matmul(ps[:], ones1[:], row[:], start=True, stop=True)
            sb = cpool.tile([128, width], dt, tag=tag)
            nc.scalar.copy(sb[:], ps[:])
            return sb

        b1_bc = bcast_row(IN["mlp1_b_row"], DM, "b1bc")
        lnw_bc = bcast_row(IN["ln_w_row"], DM, "lnwbc")
        lnb_bc = bcast_row(IN["ln_b_row"], DM, "lnbbc")

        # w1sum[dm] = sum_ps mlp1_wT -> broadcast tile
        ps_w1 = mm_tile([DM, 1])
        nc.tensor.matmul(ps_w1[:], w["mlp1_wT"][:], ones16[:], start=True, stop=True)
        w1s_col = work.tile([DM, 1], dt, tag="w1c")
        nc.vector.tensor_copy(w1s_col[:], ps_w1[:])
        ps_w1r = mm_tile([1, DM])
        nc.tensor.transpose(ps_w1r[:], w1s_col[:], ident[:])
        w1s_row = work.tile([1, DM], dt, tag="w1r")
        nc.vector.tensor_copy(w1s_row[:], ps_w1r[:])
        ps_w1b = mm_tile([128, DM])
        nc.tensor.matmul(ps_w1b[:], ones1[:], w1s_row[:], start=True, stop=True)
        w1s_bc = cpool.tile([128, DM], dt)
        nc.scalar.copy(w1s_bc[:], ps_w1b[:])

        # ---- stage A: RevIN stats ----
        sumx = spool.tile([RPC, 1], dt)
        nc.vector.reduce_sum(sumx[:], xr[:], axis=AX.X)
        mean = spool.tile([RPC, 1], dt)
        nc.vector.tensor_scalar_mul(mean[:], sumx[:], 1.0 / T)
        sq = work.tile([RPC, T], dt, tag="sq", bufs=1)
        sumx2 = spool.tile([RPC, 1], dt)
        nc.scalar.activation(sq[:], xr[:], AF.Square, accum_out=sumx2[:])
        ex2 = spool.tile([RPC, 1], dt)
        nc.vector.tensor_scalar_mul(ex2[:], sumx2[:], 1.0 / T)
        msq = spool.tile([RPC, 1], dt)
        nc.vector.tensor_mul(msq[:], mean[:], mean[:])
        var = spool.tile([RPC, 1], dt)
        nc.vector.tensor_sub(var[:], ex2[:], msq[:])
        lnv = spool.tile([RPC, 1], dt)
        nc.scalar.activation(lnv[:], var[:], AF.Ln, bias=epsc[0:RPC, :])
        std = spool.tile([RPC, 1], dt)
        nc.scalar.activation(std[:], lnv[:], AF.Exp, scale=0.5)
        istd = spool.tile([RPC, 1], dt)
        nc.scalar.activation(istd[:], lnv[:], AF.Exp, scale=-0.5)

        s_n = spool.tile([RPC, 1], dt)
        nc.vector.tensor_mul(s_n[:], wv[:], istd[:])
        o_n0 = spool.tile([RPC, 1], dt)
        nc.vector.scalar_tensor_tensor(o_n0[:], mean[:], -1.0, s_n[:],
                                       op0=OP.mult, op1=OP.mult)
        o_n = spool.tile([RPC, 1], dt)
        nc.vector.tensor_add(o_n[:], o_n0[:], bv[:])

        wq = spool.tile([RPC, 1], dt)
        nc.vector.tensor_scalar_add(wq[:], wv[:], EPS * EPS)
        rw = spool.tile([RPC, 1], dt)
        nc.vector.reciprocal(rw[:], wq[:])
        t_den = spool.tile([RPC, 1], dt)
        nc.vector.tensor_mul(t_den[:], std[:], rw[:])
        u_den0 = spool.tile([RPC, 1], dt)
        nc.vector.scalar_tensor_tensor(u_den0[:], bv[:], -1.0, t_den[:],
                                       op0=OP.mult, op1=OP.mult)
        u_den = spool.tile([RPC, 1], dt)
        nc.vector.tensor_add(u_den[:], u_den0[:], mean[:])

        svec = spool.tile([RPC, 2], dt)
        nc.vector.tensor_copy(svec[:, 0:1], s_n[:])
        nc.vector.tensor_copy(svec[:, 1:2], o_n[:])
        if DEBUG:
            stats = spool.tile([RPC, 6], dt)
            for i, tl in enumerate([mean, std, s_n, o_n, t_den, u_den]):
                nc.vector.tensor_copy(stats[:, i:i + 1], tl[:])
            dma(dbg["d_stats"], stats[:])

        # ---- stage B: mlp1 + external attention + LN + gelu + residual ----
        # structured as function-grouped passes to avoid ACT table thrash
        hT = big.tile([DM, NROW], dt, tag="hT")
        hbT = big.tile([DM, NROW], dt, tag="hbT")
        hrow_all = big.tile([128, NRT, DM], dt, tag="sluz0")
        an_all = big.tile([128, NRT, DM], dt, tag="sluz1")
        exp_all = [big.tile([128, NROW], dt, tag=tg, name=f"exp_all{i}")
                   for i, tg in enumerate(["xcT0", "xcT1", "xc2T0", "xc2T1"])]

        # B1: mlp1 + revin fold + transpose -> hT, hrow_all
        for rt in range(NRT):
            cs = rt * 128
            so_row = work.tile([128, 2], dt, tag="so_row")
            dma(so_row[:],
                svec[rt * 2:rt * 2 + 2, :].unsqueeze(1).broadcast_to((2, PN, 2)))
            patt = work.tile([PS, 128], dt, tag="patt")
            dma(patt[:], IN["patT"][:, cs:cs + 128])
            ps_h = mm_tile([128, DM])
            nc.tensor.matmul(ps_h[:], patt[:], w["mlp1_wT"][:],
                             start=True, stop=True)
            t1 = work.tile([128, DM], dt, tag="t1")
            nc.vector.scalar_tensor_tensor(t1[:], w1s_bc[:], so_row[:, 1:2],
                                           b1_bc[:], op0=OP.mult, op1=OP.add)
            nc.vector.scalar_tensor_tensor(hrow_all[:, rt, :], ps_h[:],
                                           so_row[:, 0:1], t1[:],
                                           op0=OP.mult, op1=OP.add)
            ps_tr = mm_tile([DM, 128])
            nc.tensor.transpose(ps_tr[:], hrow_all[:, rt, :], ident[:])
            nc.scalar.copy(hT[:, cs:cs + 128], ps_tr[:])

        # B2: logits + exp (exp table)
        for rt in range(NRT):
            cs = rt * 128
            for sc in range(4):
                ps_l = mm_tile([128, 128])
                nc.tensor.matmul(ps_l[:], w["mk_wT"][:, sc * 128:(sc + 1) * 128],
                                 hT[:, cs:cs + 128], start=True, stop=True)
                nc.scalar.activation(exp_all[sc][:, cs:cs + 128], ps_l[:], AF.Exp)

        # B3: attnv (+sum column) + normalize
        for rt in range(NRT):
            cs = rt * 128
            ps_at = ps_y.tile([128, DM + 1], dt, tag="ps_y0", name="ps_at")
            for sc in range(4):
                nc.tensor.matmul(ps_at[:], exp_all[sc][:, cs:cs + 128],
                                 mv_aug[:, sc * 129:(sc + 1) * 129],
                                 start=(sc == 0), stop=(sc == 3))
            rec = work.tile([128, 1], dt, tag="rec")
            nc.vector.reciprocal(rec[:], ps_at[:, DM:DM + 1])
            nc.vector.tensor_scalar_mul(an_all[:, rt, :], ps_at[:, 0:DM], rec[:])

        # B4a: LN stats for all row tiles (Square is in every act table)
        mu_all = spool.tile([128, NRT], dt)
        varr_all = spool.tile([128, NRT], dt)
        for rt in range(NRT):
            a_n = an_all[:, rt, :]
            sm = work.tile([128, 1], dt, tag="sm")
            nc.vector.reduce_sum(sm[:], a_n, axis=AX.X)
            nc.vector.tensor_scalar_mul(mu_all[:, rt:rt + 1], sm[:], 1.0 / DM)
            sqs = work2.tile([128, DM], dt, tag="sqs")
            ssq = work.tile([128, 1], dt, tag="ssq")
            nc.scalar.activation(sqs[:], a_n, AF.Square, accum_out=ssq[:])
            ex2r = work.tile([128, 1], dt, tag="ex2r")
            nc.vector.tensor_scalar_mul(ex2r[:], ssq[:], 1.0 / DM)
            msqr = work.tile([128, 1], dt, tag="msqr")
            nc.vector.tensor_mul(msqr[:], mu_all[:, rt:rt + 1],
                                 mu_all[:, rt:rt + 1])
            nc.vector.tensor_sub(varr_all[:, rt:rt + 1], ex2r[:], msqr[:])
        # B4b: one Ln + one Exp for all tiles (single table switch each)
        lnr_all = spool.tile([128, NRT], dt)
        nc.scalar.activation(lnr_all[:], varr_all[:], AF.Ln, bias=epsc[:])
        rstd_all = spool.tile([128, NRT], dt)
        i_rstd = nc.scalar.activation(rstd_all[:], lnr_all[:], AF.Exp,
                                      scale=-0.5)
        last_b4_act = i_rstd
        m2_all = spool.tile([128, NRT], dt)
        nc.vector.scalar_tensor_tensor(m2_all[:], mu_all[:], -1.0, rstd_all[:],
                                       op0=OP.mult, op1=OP.mult)
        # B4c: normalize + ln scale/shift
        for rt in range(NRT):
            a_n = an_all[:, rt, :]
            q = work2.tile([128, DM], dt, tag="q")
            nc.vector.tensor_scalar(q[:], a_n, rstd_all[:, rt:rt + 1],
                                    m2_all[:, rt:rt + 1],
                                    op0=OP.mult, op1=OP.add)
            ln = work2.tile([128, DM], dt, tag="ln")
            nc.vector.tensor_mul(ln[:], q[:], lnw_bc[:])
            nc.vector.tensor_add(an_all[:, rt, :], ln[:], lnb_bc[:])

        # B5: gelu + residual + transpose -> hbT (gelu table)
        last_gelu = None
        for rt in range(NRT):
            cs = rt * 128
            g = work2.tile([128, DM], dt, tag="g")
            i_g = nc.scalar.activation(g[:], an_all[:, rt, :], AF.Gelu)
            if rt == 0:
                add_dep_helper(i_g.ins, last_b4_act.ins, sync=True,
                               reason="act table: gelu after nle")
            last_gelu = i_g
            hb_row = work2.tile([128, DM], dt, tag="hb_row")
            nc.vector.tensor_add(hb_row[:], g[:], hrow_all[:, rt, :])
            ps_tb = mm_tile([DM, 128])
            nc.tensor.transpose(ps_tb[:], hb_row[:], ident[:])
            nc.scalar.copy(hbT[:, cs:cs + 128], ps_tb[:])

        if DEBUG:
            dma(dbg["d_hT"], hT[:])
            dma(dbg["d_hbT"], hbT[:])

        # ---- stage D: in_proj -> xcT (padded); z -> silu_z (padded) ----
        xcT = [big.tile([128, LT], dt, tag=f"xcT{cc}", name=f"xcT{cc}") for cc in range(2)]
        sluz = [big.tile([128, NROW], dt, tag=f"sluz{cc}", name=f"sluz{cc}") for cc in range(2)]
        for cc in range(2):
            # only the gap columns need zeroing (conv taps read them)
            nc.gpsimd.memset(
                xcT[cc][:].rearrange("p (b l) -> p b l", b=RPC)[:, :, 0:GAP], 0.0)
        ccw = [(i * 512, min(512, NROW - i * 512))
               for i in range((NROW + 511) // 512)]
        first_silu = None
        for pc in range(4):
            cchunk, isx = (pc % 2), (pc < 2)
            for (c0, cw) in ccw:
                nblk_c = cw // PN
                ps_x = mm_tile([128, 512])
                nc.tensor.matmul(ps_x[:, :cw],
                                 w["in_proj_wT"][:, pc * 128:(pc + 1) * 128],
                                 hbT[:, c0:c0 + cw], start=True, stop=True)
                if isx:
                    p0 = (c0 // PN) * LP
                    dview = xcT[cchunk][:, p0:p0 + nblk_c * LP].rearrange(
                        "p (b l) -> p b l", b=nblk_c)[:, :, GAP:LP]
                    sview = ps_x[:, :cw].rearrange("p (b l) -> p b l", b=nblk_c)
                    nc.vector.tensor_copy(dview, sview)
                else:
                    i_s = nc.scalar.activation(sluz[cchunk][:, c0:c0 + cw],
                                               ps_x[:, :cw], AF.Silu)
                    if first_silu is None:
                        first_silu = i_s
                        add_dep_helper(i_s.ins, last_gelu.ins, sync=True,
                                       reason="act table: silu after gelu")

        # ---- stage E: causal depthwise conv + silu (chunked, no in-place) ----
        xc2T = [big.tile([128, LT], dt, tag=f"xc2T{cc}", name=f"xc2T{cc}")
                for cc in range(2)]
        for cc in range(2):
            nc.gpsimd.memset(
                xc2T[cc][:].rearrange("p (b l) -> p b l", b=RPC)[:, :, 0:GAP],
                0.0)
            wsl = w["conv_w2"][cc]
            for si in range(NSC):
                c0 = si * SCW
                cw_ = SCW - GAP
                t1c = scanp.tile([128, cw_], dt, tag="a_t", name="cv1")
                nc.vector.tensor_scalar(t1c[:], xcT[cc][:, c0:c0 + cw_],
                                        wsl[:, 0:1], None, op0=OP.mult)
                t2c = scanp.tile([128, cw_], dt, tag="b_t", name="cv2")
                nc.vector.scalar_tensor_tensor(t2c[:],
                                               xcT[cc][:, c0 + 1:c0 + 1 + cw_],
                                               wsl[:, 1:2], t1c[:],
                                               op0=OP.mult, op1=OP.add)
                t3c = scanp.tile([128, cw_], dt, tag="a_t", name="t3c")
                nc.vector.scalar_tensor_tensor(t3c[:],
                                               xcT[cc][:, c0 + 2:c0 + 2 + cw_],
                                               wsl[:, 2:3], t2c[:],
                                               op0=OP.mult, op1=OP.add)
                t4c = scanp.tile([128, cw_], dt, tag="b_t", name="t4c")
                nc.vector.scalar_tensor_tensor(t4c[:],
                                               xcT[cc][:, c0 + 3:c0 + 3 + cw_],
                                               wsl[:, 3:4], t3c[:],
                                               op0=OP.mult, op1=OP.add)
                i_cs = nc.scalar.activation(xc2T[cc][:, c0 + GAP:c0 + SCW],
                                            t4c[:], AF.Silu,
                                            bias=w["conv_b_col"][cc][:])
                last_silu = i_cs
        if DEBUG:
            for cc in range(2):
                dma(dbg["d_xc2T"][cc * 128:(cc + 1) * 128, :], xc2T[cc][:])

        # ---- stage F: x_proj -> (dt,Bm,Cm); dt_proj -> delta; du ----
        # separate tiles so each starts at partition 0 (matmul base rule)
        dblD = big.tile([DTR, LT], dt, tag="hT")  # reuse hT slot (dead)
        dblB_t = big.tile([DS, LT], mybir.dt.bfloat16, tag="dblB")
        dblC_t = big.tile([DS, LT], mybir.dt.bfloat16, tag="dblC")
        dblB = dblB_t[:]
        dblC = dblC_t[:]
        for si in range(NSC):
            c0 = si * SCW
            for (lo, hi, dst) in [(0, DTR, dblD[:]), (DTR, DTR + DS, dblB),
                                  (DTR + DS, DTR + 2 * DS, dblC)]:
                ps_d = mm_tile([hi - lo, SCW])
                for cc in range(2):
                    nc.tensor.matmul(ps_d[:],
                                     w["x_proj_wT"][cc][:, lo:hi],
                                     xc2T[cc][:, c0:c0 + SCW],
                                     start=(cc == 0), stop=(cc == 1))
                nc.scalar.copy(dst[:, c0:c0 + SCW], ps_d[:])
        if DEBUG:
            dma(dbg["d_dblT"][0:DTR, :], dblD[:])
            dma(dbg["d_dblB"], dblB)
            dma(dbg["d_dblC"], dblC)

        deltaT = [big.tile([128, LT], dt, tag=f"xcT{cc}", name=f"deltaT{cc}") for cc in range(2)]
        duT = [big.tile([128, LT], dt, tag=t, name=f"duT_{t}") for t in ("convacc", "hbT")]
        # Exp pass (staged into duT), then Ln pass -> softplus, grouped so the
        # act-table switches once per function
        for cc in range(2):
            for si in range(NSC):
                c0 = si * SCW
                ps_dt = mm_tile([128, SCW])
                nc.tensor.matmul(ps_dt[:],
                                 w["dt_proj_wT"][:, cc * 128:(cc + 1) * 128],
                                 dblD[:][:, c0:c0 + SCW], start=True, stop=True)
                i_e1 = nc.scalar.activation(duT[cc][:, c0:c0 + SCW], ps_dt[:],
                                            AF.Exp,
                                            bias=w["dt_proj_b_col"][cc][:])
                if cc == 0 and si == 0:
                    add_dep_helper(i_e1.ins, last_silu.ins, sync=True,
                                   reason="act table: exp after silu")
                last_exp_f = i_e1
        first_agen = None
        last_softplus = None
        for cc in range(2):
            for si in range(NSC):
                c0 = si * SCW
                i_ln = nc.scalar.activation(deltaT[cc][:, c0:c0 + SCW],
                                            duT[cc][:, c0:c0 + SCW],
                                            AF.Ln, bias=1.0)
                add_dep_helper(i_ln.ins, last_exp_f.ins, sync=True,
                               reason="act table: ln after exp pass")
                last_softplus = i_ln
                dv = lambda t: t[:, c0:c0 + SCW].rearrange(
                    "p (b l) -> p b l", b=SCB)
                nc.gpsimd.memset(dv(duT[cc])[:, :, 0:GAP], 0.0)
                nc.vector.tensor_mul(dv(duT[cc])[:, :, GAP:LP],
                                     dv(deltaT[cc])[:, :, GAP:LP],
                                     dv(xc2T[cc])[:, :, GAP:LP])
                # poison delta gaps so exp(A*delta)=0 there (state reset)
                nc.vector.memset(dv(deltaT[cc])[:, :, 0:GAP], POISON)
        if DEBUG:
            for cc in range(2):
                dma(dbg["d_deltaT"][cc * 128:(cc + 1) * 128, :], deltaT[cc][:])
                dma(dbg["d_duT"][cc * 128:(cc + 1) * 128, :], duT[cc][:])

        # ---- stage G: selective scan ----
        # one-hot row-selection matrix: sel[i, d*128+m] = (i == d)
        sel = cpool.tile([DS, DS * 128], mybir.dt.bfloat16)
        nc.gpsimd.memset(sel[:], 0.0)
        nc.gpsimd.affine_select(out=sel[:], in_=sel[:],
                                compare_op=OP.not_equal, fill=1.0,
                                base=0, pattern=[[-1, DS], [0, 128]],
                                channel_multiplier=1)
        y2T = [big.tile([128, NROW], dt, tag=f"y2T{cc}", name=f"y2T{cc}") for cc in range(2)]

        for si in range(NSC):
            c0 = si * SCW
            ps_ys = [ps_y.tile([128, SCW], dt, tag=f"ps_y{cc}", name=f"ps_ys{cc}")
                     for cc in range(2)]
            for d in range(DS):
                ps_bm = ps_bc.tile([128, SCW], dt, tag="ps_bm")
                nc.tensor.matmul(ps_bm[:],
                                 sel[:, d * 128:(d + 1) * 128],
                                 dblB[:, c0:c0 + SCW],
                                 start=True, stop=True)
                ps_cm = ps_bc.tile([128, SCW], dt, tag="ps_cm")
                nc.tensor.matmul(ps_cm[:],
                                 sel[:, d * 128:(d + 1) * 128],
                                 dblC[:, c0:c0 + SCW],
                                 start=True, stop=True)
                for cc in range(2):
                    a_t = scanp.tile([128, SCW], mybir.dt.bfloat16, tag="sc_a",
                                     name="a_t")
                    i_ag = nc.scalar.activation(a_t[:],
                                                deltaT[cc][:, c0:c0 + SCW],
                                                AF.Exp,
                                                scale=A_sb[cc][:, d:d + 1])
                    if first_agen is None:
                        first_agen = i_ag
                        add_dep_helper(i_ag.ins, last_softplus.ins, sync=True,
                                       reason="act table: exp after ln")
                    b_t = scanp.tile([128, SCW], mybir.dt.bfloat16, tag="sc_b",
                                     name="b_t")
                    nc.vector.tensor_mul(b_t[:], duT[cc][:, c0:c0 + SCW],
                                         ps_bm[:])
                    h_t = scanp.tile([128, SCW], mybir.dt.bfloat16, tag="h_t")
                    nc.vector.tensor_tensor_scan(
                        h_t[:], a_t[:], b_t[:], initial=0.0,
                        op0=OP.mult, op1=OP.add)
                    p_t = scanp.tile([128, SCW], mybir.dt.bfloat16, tag="p_t")
                    nc.vector.tensor_mul(p_t[:], h_t[:], ps_cm[:])
                    nc.tensor.matmul(ps_ys[cc][:], identb[:], p_t[:],
                                     start=(d == 0), stop=(d == DS - 1))
            d0 = si * SCB * PN
            for cc in range(2):
                t1s = scanp.tile([128, SCW], dt, tag="t1s")
                nc.vector.scalar_tensor_tensor(
                    t1s[:], xc2T[cc][:, c0:c0 + SCW],
                    w["D_col"][cc][:], ps_ys[cc][:],
                    op0=OP.mult, op1=OP.add)
                nc.vector.tensor_mul(
                    y2T[cc][:, d0:d0 + SCB * PN].rearrange(
                        "p (b l) -> p b l", b=SCB),
                    t1s[:].rearrange("p (b l) -> p b l", b=SCB)[:, :, GAP:LP],
                    sluz[cc][:, d0:d0 + SCB * PN].rearrange(
                        "p (b l) -> p b l", b=SCB))
        if DEBUG:
            for cc in range(2):
                dma(dbg["d_y2T"][cc * 128:(cc + 1) * 128, :], y2T[cc][:])

        # ---- stage H: out_proj (compact, bf16 out) ----
        moT = big.tile([DM, NROW], mybir.dt.bfloat16, tag="sluz0", name="moT")
        CW = SCB * PN
        for si in range(NSC):
            d0 = si * CW
            ps_mo = mm_tile([DM, CW])
            for cc in range(2):
                nc.tensor.matmul(ps_mo[:],
                                 w["out_proj_wT"][cc][:],
                                 y2T[cc][:, d0:d0 + CW],
                                 start=(cc == 0), stop=(cc == 1))
            nc.scalar.copy(moT[:, d0:d0 + CW], ps_mo[:])
        if DEBUG:
            dma(dbg["d_moT"], moT[:])
        scanp_cm.__exit__(None, None, None)

        # ---- stage I: mlp2 (gelu) + mlp3 + denorm + output ----
        w2pool = P(name="w2p", bufs=1)
        w2sb = w2pool.tile([DM, PN * 2 * PRED], mybir.dt.bfloat16)
        dma(w2sb[:], IN["w2s"])
        w2v = w2sb[:].rearrange("p (n j) -> p n j", n=PN)
        ps_o2 = ps_y.tile([128, RPC], dt, tag="ps_y0")
        ps_o2b = ps_y.tile([2 * PRED - 128, RPC], dt, tag="ps_y1")
        mo_v = moT[:].rearrange("p (b l) -> p b l", b=RPC)
        for pn in range(PN):
            rhs = mo_v[:, :, pn:pn + 1]
            nc.tensor.matmul(ps_o2[:], w2v[:, pn, 0:128], rhs,
                             start=(pn == 0), stop=(pn == PN - 1))
            nc.tensor.matmul(ps_o2b[:], w2v[:, pn, 128:2 * PRED], rhs,
                             start=(pn == 0), stop=(pn == PN - 1))
        o2a = work.tile([128, RPC], dt, tag="o2a")
        nc.scalar.activation(o2a[:], ps_o2[:], AF.Gelu,
                             bias=w["mlp2_b_col"][0][:])
        o2b = work.tile([2 * PRED - 128, RPC], dt, tag="o2b")
        nc.scalar.activation(o2b[:], ps_o2b[:], AF.Gelu,
                             bias=w["mlp2_b_col"][1][:])
        ps_o3 = mm_tile([PRED, RPC])
        nc.tensor.matmul(ps_o3[:], w["mlp3_wT"][0][:], o2a[:],
                         start=True, stop=False)
        nc.tensor.matmul(ps_o3[:], w["mlp3_wT"][1][:], o2b[:],
                         start=False, stop=True)
        o3T = work.tile([PRED, RPC], dt, tag="o3T")
        nc.vector.tensor_copy(o3T[:], ps_o3[:])
        ps_o3t = mm_tile([RPC, PRED])
        nc.tensor.transpose(ps_o3t[:], o3T[:], ident[0:PRED, 0:PRED])

        b3row = work.tile([1, PRED], dt, tag="b3row")
        dma(b3row[:], IN["mlp3_b_row"])
        ps_b3 = mm_tile([RPC, PRED])
        nc.tensor.matmul(ps_b3[:], ones1[:, 0:RPC], b3row[:],
                         start=True, stop=True)
        den = work.tile([RPC, PRED], dt, tag="den")
        nc.vector.tensor_scalar(den[:], ps_b3[:], t_den[:], u_den[:],
                                op0=OP.mult, op1=OP.add)
        out_sb = work.tile([RPC, PRED], dt, tag="out_sb")
        nc.vector.scalar_tensor_tensor(out_sb[:], ps_o3t[:], t_den[:], den[:],
                                       op0=OP.mult, op1=OP.add)
        dma(out_d, out_sb[:])

    nc.compile()
    _cache[key] = nc
    return nc


def kernel(**inputs):
    nc = build_program()
    in_maps = prep_inputs(inputs)
    res = run_bass_kernel_spmd(nc, in_maps, list(range(NCORES)))
    return assemble(res.results)


if __name__ == "__main__":
    import reference as R
    inp = R.setup_inputs()
    out = kernel(**{k: np.asarray(v) for k, v in inp.items()})
    print("kernel out", out.shape, out.dtype, np.abs(out).max())



# revision 2
# speedup vs baseline: 5.6163x; 5.6163x over previous
"""Trainium2 Bass kernel for nn_Backbone1_62947040690721.

Data-parallel over the fused B*NV block axis: 336 independent per-series
problems, 42 per NeuronCore across 8 cores. All weights replicated.

Layouts (per core, 42 blocks):
  - row layout: (block,patch) rows on partitions (128 rows = 2 blocks/tile)
  - T layout:   features on partitions, l = concat of blocks on the free axis,
                padded to 67 cols/block (3 zero "gap" cols + 64 data cols).
                The gaps give causal-conv zero padding and selective-scan
                state resets between blocks for free.
The selective scan runs as hardware tensor_tensor_scan instructions, one per
(state dim d, channel chunk), scanning 7 blocks' timelines per call; scan
chunks are block-aligned so every chunk self-resets at its leading gap cols.
"""

import sys

sys.path.insert(0, "/opt/trn_rl_repo")

import numpy as np

import concourse.bass as bass
import concourse.mybir as mybir
import concourse.tile as tile
from concourse import bacc
from concourse.bass_utils import run_bass_kernel_spmd

F32 = mybir.dt.float32
F32R = mybir.dt.float32r
AF = mybir.ActivationFunctionType
OP = mybir.AluOpType
AX = mybir.AxisListType

# model dims
B, T, NV = 16, 512, 21
PS, STRIDE, PRED = 16, 8, 96
DM, DS, DC = 128, 16, 4
DIN = 2 * DM          # 256
DTR = 8
S_EA = 512
PN = (T - PS) // STRIDE + 1 + 1  # 64
EPS = 1e-5

NCORES = 8
NBLK = B * NV          # 336
RPC = NBLK // NCORES   # 42 blocks per core
NROW = RPC * PN        # 2688 compact rows per core
NRT = NROW // 128      # 21 row tiles
GAP = 3                # zero-pad cols before each block
LP = PN + GAP          # 67 padded cols per block
LT = RPC * LP          # 2814 padded timeline length
SCB = 7                # blocks per scan chunk
SCW = SCB * LP         # 469 scan chunk width (>=256 keeps fp32r full rate)
NSC = RPC // SCB       # 6 scan chunks
POISON = 1.0e30

DEBUG = False          # set True (before first kernel build) for stage taps
REPEAT = 2             # build the body N times (for differential timing)

_cache = {}


def _r(x):
    return np.ascontiguousarray(np.asarray(x, dtype=np.float32))


def prep_inputs(inputs):
    """Full inputs -> per-core input maps (pure data movement on host)."""
    x = _r(inputs["x"])
    xbn = np.ascontiguousarray(x.transpose(0, 2, 1).reshape(NBLK, T))
    xp = np.concatenate([xbn, np.repeat(xbn[:, -1:], STRIDE, axis=1)], axis=1)
    idx = np.arange(PN)[:, None] * STRIDE + np.arange(PS)[None, :]
    pat = xp[:, idx]                                     # (336, 64, 16)
    patT = np.ascontiguousarray(pat.transpose(2, 0, 1))  # (16, 336, 64)
    wv = np.tile(_r(inputs["revin_w"]), B).reshape(NBLK, 1)
    bv = np.tile(_r(inputs["revin_b"]), B).reshape(NBLK, 1)

    import ml_dtypes
    mlp2_wT = _r(inputs["mlp2_w"]).T       # (8192, 192)
    w2s = np.ascontiguousarray(
        mlp2_wT.reshape(PN, DM, 2 * PRED).transpose(1, 0, 2)
    ).astype(ml_dtypes.bfloat16)  # (128, 64, 192) bf16

    shared = {
        "mlp1_wT": _r(inputs["mlp1_w"]).T.copy(),          # (16,128)
        "mlp1_b_row": _r(inputs["mlp1_b"]).reshape(1, DM),
        "mk_wT": _r(inputs["mk_w"]).T.copy(),              # (128,512)
        "mv_wT": _r(inputs["mv_w"]).T.copy(),              # (512,128)
        "ln_w_row": _r(inputs["ln_w"]).reshape(1, DM),
        "ln_b_row": _r(inputs["ln_b"]).reshape(1, DM),
        "in_proj_wT": _r(inputs["in_proj_w"]).T.copy(),    # (128,512)
        "conv_w2": _r(inputs["conv_w"])[:, 0, :].copy(),   # (256,4)
        "conv_b_col": _r(inputs["conv_b"]).reshape(DIN, 1),
        "x_proj_wT": _r(inputs["x_proj_w"]).T.copy(),      # (256,40)
        "dt_proj_wT": _r(inputs["dt_proj_w"]).T.copy(),    # (8,256)
        "dt_proj_b_col": _r(inputs["dt_proj_b"]).reshape(DIN, 1),
        "A_log_in": _r(inputs["A_log"]),                   # (256,16)
        "D_col": _r(inputs["D_ssm"]).reshape(DIN, 1),
        "out_proj_wT": _r(inputs["out_proj_w"]).T.copy(),  # (256,128)
        "w2s": w2s,                                        # (128,64,192)
        "mlp2_b_col": _r(inputs["mlp2_b"]).reshape(2 * PRED, 1),
        "mlp3_wT": _r(inputs["mlp3_w"]).T.copy(),          # (192,96)
        "mlp3_b_row": _r(inputs["mlp3_b"]).reshape(1, PRED),
    }
    in_maps = []
    for c in range(NCORES):
        lo, hi = c * RPC, (c + 1) * RPC
        m = dict(shared)
        m["xrow"] = np.ascontiguousarray(xbn[lo:hi])                  # (42,512)
        m["patT"] = np.ascontiguousarray(patT[:, lo:hi, :]).reshape(PS, NROW)
        m["wv"] = np.ascontiguousarray(wv[lo:hi])
        m["bv"] = np.ascontiguousarray(bv[lo:hi])
        in_maps.append(m)
    return in_maps


def assemble(results):
    outs = np.concatenate([r["out"] for r in results], axis=0)  # (336, 96)
    out = outs.reshape(B, NV, PRED).transpose(0, 2, 1)
    return np.ascontiguousarray(out.astype(np.float32))


# ---------------------------------------------------------------------------
# program builder
# ---------------------------------------------------------------------------

def _decl_inputs(nc):
    d = {}
    spec = {
        "xrow": (RPC, T), "patT": (PS, NROW), "wv": (RPC, 1), "bv": (RPC, 1),
        "mlp1_wT": (PS, DM), "mlp1_b_row": (1, DM),
        "mk_wT": (DM, S_EA), "mv_wT": (S_EA, DM),
        "ln_w_row": (1, DM), "ln_b_row": (1, DM),
        "in_proj_wT": (DM, 2 * DIN),
        "conv_w2": (DIN, DC), "conv_b_col": (DIN, 1),
        "x_proj_wT": (DIN, DTR + 2 * DS),
        "dt_proj_wT": (DTR, DIN), "dt_proj_b_col": (DIN, 1),
        "A_log_in": (DIN, DS), "D_col": (DIN, 1),
        "out_proj_wT": (DIN, DM),
        "w2s": (DM, PN, 2 * PRED), "mlp2_b_col": (2 * PRED, 1),
        "mlp3_wT": (2 * PRED, PRED), "mlp3_b_row": (1, PRED),
    }
    for name, shape in spec.items():
        dty = mybir.dt.bfloat16 if name == "w2s" else F32
        d[name] = nc.dram_tensor(name, list(shape), dty,
                                 kind="ExternalInput").ap()
    return d


def build_program():
    key = ("nc", REPEAT, DEBUG)
    if key in _cache:
        return _cache[key]
    nc = bacc.Bacc("TRN2", target_bir_lowering=False, debug=False,
                   num_devices=NCORES)
    IN = _decl_inputs(nc)
    out_d = nc.dram_tensor("out", [RPC, PRED], F32, kind="ExternalOutput").ap()

    dbg = {}
    if DEBUG:
        for name, shape in [
            ("d_hT", (DM, NROW)), ("d_hbT", (DM, NROW)),
            ("d_xc2T", (DIN, LT)), ("d_deltaT", (DIN, LT)),
            ("d_duT", (DIN, LT)), ("d_y2T", (DIN, NROW)),
            ("d_moT", (DM, NROW)), ("d_dblT", (DTR + 2 * DS, LT)),
            ("d_dblB", (DS, LT)), ("d_dblC", (DS, LT)),
            ("d_stats", (RPC, 6)),
        ]:
            dty = (mybir.dt.bfloat16
                   if name in ("d_dblB", "d_dblC", "d_moT") else F32)
            dbg[name] = nc.dram_tensor(name, list(shape), dty,
                                       kind="ExternalOutput").ap()

    from contextlib import ExitStack
    from concourse.masks import make_identity
    from concourse.tile import add_dep_helper

    with tile.TileContext(nc) as tc:
      for _rep in range(REPEAT):
       with ExitStack() as ctx:
        P = lambda **kw: ctx.enter_context(tc.tile_pool(**kw))
        wpool = P(name="weights", bufs=1)
        cpool = P(name="consts", bufs=1)
        spool = P(name="statp", bufs=1)
        big = P(name="bigact", bufs=1)
        work = P(name="work", bufs=2)
        work2 = P(name="work2", bufs=2)
        scanp_cm = tc.tile_pool(name="scanp", bufs=3)
        scanp = scanp_cm.__enter__()
        # PSUM: mm(2 banks) + bc(4 banks) + y(2 banks) = 8 banks
        ps_mm = P(name="ps_mm", bufs=2, space="PSUM")
        ps_bc = P(name="ps_bc", bufs=2, space="PSUM")
        ps_y = P(name="ps_y", bufs=1, space="PSUM")

        dt = F32

        def dma(dst, src):
            nc.sync.dma_start(out=dst, in_=src)

        def mm_tile(shape, tag="mm"):
            return ps_mm.tile(list(shape), dt, tag=tag, name=tag)

        # ---- constants / weights to SBUF ----
        ident = cpool.tile([128, 128], dt)
        make_identity(nc, ident[:])
        identb = cpool.tile([128, 128], mybir.dt.bfloat16)
        make_identity(nc, identb[:])
        ones1 = cpool.tile([1, 128], dt)
        nc.vector.memset(ones1[:], 1.0)
        ones16 = cpool.tile([PS, 1], dt)
        nc.vector.memset(ones16[:], 1.0)
        epsc = cpool.tile([128, 1], dt)
        nc.vector.memset(epsc[:], EPS)

        # input data first so stage A starts immediately
        xr = big.tile([RPC, T], dt, tag="xrow")
        dma(xr[:], IN["xrow"])
        wv = spool.tile([RPC, 1], dt)
        dma(wv[:], IN["wv"])
        bv = spool.tile([RPC, 1], dt)
        dma(bv[:], IN["bv"])

        w = {}
        for name, shape in [
            ("mlp1_wT", (PS, DM)), ("mk_wT", (DM, S_EA)),
            ("in_proj_wT", (DM, 2 * DIN)), ("dt_proj_wT", (DTR, DIN)),
        ]:
            tl = wpool.tile(list(shape), dt, tag=name)
            dma(tl[:], IN[name])
            w[name] = tl
        # channel-chunked weights (DIN=256 or 192 rows -> per-128 tiles)
        for name, shape in [
            ("conv_w2", (DIN, DC)), ("conv_b_col", (DIN, 1)),
            ("x_proj_wT", (DIN, DTR + 2 * DS)), ("dt_proj_b_col", (DIN, 1)),
            ("D_col", (DIN, 1)), ("out_proj_wT", (DIN, DM)),
            ("mlp2_b_col", (2 * PRED, 1)), ("mlp3_wT", (2 * PRED, PRED)),
        ]:
            rows = shape[0]
            parts = []
            for cc in range((rows + 127) // 128):
                r0 = cc * 128
                r1 = min(rows, r0 + 128)
                tl = wpool.tile([r1 - r0, shape[1]], dt, tag=f"{name}{cc}")
                dma(tl[:], IN[name][r0:r1, :])
                parts.append(tl)
            w[name] = parts

        # mv_aug: mv_wT chunks with an appended ones column (for softmax sums)
        mv_aug = wpool.tile([128, 4 * (DM + 1)], dt)
        for sc in range(4):
            dma(mv_aug[:, sc * 129:sc * 129 + DM],
                IN["mv_wT"][sc * 128:(sc + 1) * 128, :])
            nc.vector.memset(mv_aug[:, sc * 129 + DM:(sc + 1) * 129], 1.0)

        # A = -exp(A_log), (128,16) per channel chunk
        A_sb = []
        for cc in range(2):
            raw = work.tile([128, DS], dt, tag="araw")
            dma(raw[:], IN["A_log_in"][cc * 128:(cc + 1) * 128, :])
            ex = work.tile([128, DS], dt, tag="aexp")
            nc.scalar.activation(ex[:], raw[:], AF.Exp)
            neg = wpool.tile([128, DS], dt, tag=f"A_{cc}")
            nc.vector.tensor_scalar_mul(neg[:], ex[:], -1.0)
            A_sb.append(neg)

        # broadcast a (1,width) DRAM row -> (128,width) SBUF tile
        def bcast_row(dram_row, width, tag):
            row = work.tile([1, width], dt, tag="brow")
            dma(row[:], dram_row)
            ps = mm_tile([128, width])
            nc.tensor.matmul(ps[:], ones1[:], row[:], start=True, stop=True)
            sb = cpool.tile([128, width], dt, tag=tag)
            nc.scalar.copy(sb[:], ps[:])
            return sb

        b1_bc = bcast_row(IN["mlp1_b_row"], DM, "b1bc")
        lnw_bc = bcast_row(IN["ln_w_row"], DM, "lnwbc")
        lnb_bc = bcast_row(IN["ln_b_row"], DM, "lnbbc")

        # w1sum[dm] = sum_ps mlp1_wT -> broadcast tile
        ps_w1 = mm_tile([DM, 1])
        nc.tensor.matmul(ps_w1[:], w["mlp1_wT"][:], ones16[:], start=True, stop=True)
        w1s_col = work.tile([DM, 1], dt, tag="w1c")
        nc.vector.tensor_copy(w1s_col[:], ps_w1[:])
        ps_w1r = mm_tile([1, DM])
        nc.tensor.transpose(ps_w1r[:], w1s_col[:], ident[:])
        w1s_row = work.tile([1, DM], dt, tag="w1r")
        nc.vector.tensor_copy(w1s_row[:], ps_w1r[:])
        ps_w1b = mm_tile([128, DM])
        nc.tensor.matmul(ps_w1b[:], ones1[:], w1s_row[:], start=True, stop=True)
        w1s_bc = cpool.tile([128, DM], dt)
        nc.scalar.copy(w1s_bc[:], ps_w1b[:])

        # ---- stage A: RevIN stats ----
        sumx = spool.tile([RPC, 1], dt)
        nc.vector.reduce_sum(sumx[:], xr[:], axis=AX.X)
        mean = spool.tile([RPC, 1], dt)
        nc.vector.tensor_scalar_mul(mean[:], sumx[:], 1.0 / T)
        sq = work.tile([RPC, T], dt, tag="sq", bufs=1)
        sumx2 = spool.tile([RPC, 1], dt)
        nc.scalar.activation(sq[:], xr[:], AF.Square, accum_out=sumx2[:])
        ex2 = spool.tile([RPC, 1], dt)
        nc.vector.tensor_scalar_mul(ex2[:], sumx2[:], 1.0 / T)
        msq = spool.tile([RPC, 1], dt)
        nc.vector.tensor_mul(msq[:], mean[:], mean[:])
        var = spool.tile([RPC, 1], dt)
        nc.vector.tensor_sub(var[:], ex2[:], msq[:])
        lnv = spool.tile([RPC, 1], dt)
        nc.scalar.activation(lnv[:], var[:], AF.Ln, bias=epsc[0:RPC, :])
        std = spool.tile([RPC, 1], dt)
        nc.scalar.activation(std[:], lnv[:], AF.Exp, scale=0.5)
        istd = spool.tile([RPC, 1], dt)
        nc.scalar.activation(istd[:], lnv[:], AF.Exp, scale=-0.5)

        s_n = spool.tile([RPC, 1], dt)
        nc.vector.tensor_mul(s_n[:], wv[:], istd[:])
        o_n0 = spool.tile([RPC, 1], dt)
        nc.vector.scalar_tensor_tensor(o_n0[:], mean[:], -1.0, s_n[:],
                                       op0=OP.mult, op1=OP.mult)
        o_n = spool.tile([RPC, 1], dt)
        nc.vector.tensor_add(o_n[:], o_n0[:], bv[:])

        wq = spool.tile([RPC, 1], dt)
        nc.vector.tensor_scalar_add(wq[:], wv[:], EPS * EPS)
        rw = spool.tile([RPC, 1], dt)
        nc.vector.reciprocal(rw[:], wq[:])
        t_den = spool.tile([RPC, 1], dt)
        nc.vector.tensor_mul(t_den[:], std[:], rw[:])
        u_den0 = spool.tile([RPC, 1], dt)
        nc.vector.scalar_tensor_tensor(u_den0[:], bv[:], -1.0, t_den[:],
                                       op0=OP.mult, op1=OP.mult)
        u_den = spool.tile([RPC, 1], dt)
        nc.vector.tensor_add(u_den[:], u_den0[:], mean[:])

        svec = spool.tile([RPC, 2], dt)
        nc.vector.tensor_copy(svec[:, 0:1], s_n[:])
        nc.vector.tensor_copy(svec[:, 1:2], o_n[:])
        if DEBUG:
            stats = spool.tile([RPC, 6], dt)
            for i, tl in enumerate([mean, std, s_n, o_n, t_den, u_den]):
                nc.vector.tensor_copy(stats[:, i:i + 1], tl[:])
            dma(dbg["d_stats"], stats[:])

        # ---- stage B: mlp1 + external attention + LN + gelu + residual ----
        # structured as function-grouped passes to avoid ACT table thrash
        hT = big.tile([DM, NROW], dt, tag="hT")
        hbT = big.tile([DM, NROW], dt, tag="hbT")
        hrow_all = big.tile([128, NRT, DM], dt, tag="sluz0")
        an_all = big.tile([128, NRT, DM], dt, tag="sluz1")
        exp_all = [big.tile([128, NROW], dt, tag=tg, name=f"exp_all{i}")
                   for i, tg in enumerate(["xcT0", "xcT1", "xc2T0", "xc2T1"])]

        # B1: mlp1 + revin fold + transpose -> hT, hrow_all
        for rt in range(NRT):
            cs = rt * 128
            so_row = work.tile([128, 2], dt, tag="so_row")
            dma(so_row[:],
                svec[rt * 2:rt * 2 + 2, :].unsqueeze(1).broadcast_to((2, PN, 2)))
            patt = work.tile([PS, 128], dt, tag="patt")
            dma(patt[:], IN["patT"][:, cs:cs + 128])
            ps_h = mm_tile([128, DM])
            nc.tensor.matmul(ps_h[:], patt[:], w["mlp1_wT"][:],
                             start=True, stop=True)
            t1 = work.tile([128, DM], dt, tag="t1")
            nc.vector.scalar_tensor_tensor(t1[:], w1s_bc[:], so_row[:, 1:2],
                                           b1_bc[:], op0=OP.mult, op1=OP.add)
            nc.vector.scalar_tensor_tensor(hrow_all[:, rt, :], ps_h[:],
                                           so_row[:, 0:1], t1[:],
                                           op0=OP.mult, op1=OP.add)
            ps_tr = mm_tile([DM, 128])
            nc.tensor.transpose(ps_tr[:], hrow_all[:, rt, :], ident[:])
            nc.scalar.copy(hT[:, cs:cs + 128], ps_tr[:])

        # B2: logits + exp (exp table)
        for rt in range(NRT):
            cs = rt * 128
            for sc in range(4):
                ps_l = mm_tile([128, 128])
                nc.tensor.matmul(ps_l[:], w["mk_wT"][:, sc * 128:(sc + 1) * 128],
                                 hT[:, cs:cs + 128], start=True, stop=True)
                nc.scalar.activation(exp_all[sc][:, cs:cs + 128], ps_l[:], AF.Exp)

        # B3: attnv (+sum column) + normalize
        for rt in range(NRT):
            cs = rt * 128
            ps_at = ps_y.tile([128, DM + 1], dt, tag="ps_y0", name="ps_at")
            for sc in range(4):
                nc.tensor.matmul(ps_at[:], exp_all[sc][:, cs:cs + 128],
                                 mv_aug[:, sc * 129:(sc + 1) * 129],
                                 start=(sc == 0), stop=(sc == 3))
            rec = work.tile([128, 1], dt, tag="rec")
            nc.vector.reciprocal(rec[:], ps_at[:, DM:DM + 1])
            nc.vector.tensor_scalar_mul(an_all[:, rt, :], ps_at[:, 0:DM], rec[:])

        # B4a: LN stats for all row tiles (Square is in every act table)
        mu_all = spool.tile([128, NRT], dt)
        varr_all = spool.tile([128, NRT], dt)
        for rt in range(NRT):
            a_n = an_all[:, rt, :]
            sm = work.tile([128, 1], dt, tag="sm")
            nc.vector.reduce_sum(sm[:], a_n, axis=AX.X)
            nc.vector.tensor_scalar_mul(mu_all[:, rt:rt + 1], sm[:], 1.0 / DM)
            sqs = work2.tile([128, DM], dt, tag="sqs")
            ssq = work.tile([128, 1], dt, tag="ssq")
            nc.scalar.activation(sqs[:], a_n, AF.Square, accum_out=ssq[:])
            ex2r = work.tile([128, 1], dt, tag="ex2r")
            nc.vector.tensor_scalar_mul(ex2r[:], ssq[:], 1.0 / DM)
            msqr = work.tile([128, 1], dt, tag="msqr")
            nc.vector.tensor_mul(msqr[:], mu_all[:, rt:rt + 1],
                                 mu_all[:, rt:rt + 1])
            nc.vector.tensor_sub(varr_all[:, rt:rt + 1], ex2r[:], msqr[:])
        # B4b: one Ln + one Exp for all tiles (single table switch each)
        lnr_all = spool.tile([128, NRT], dt)
        nc.scalar.activation(lnr_all[:], varr_all[:], AF.Ln, bias=epsc[:])
        rstd_all = spool.tile([128, NRT], dt)
        i_rstd = nc.scalar.activation(rstd_all[:], lnr_all[:], AF.Exp,
                                      scale=-0.5)
        last_b4_act = i_rstd
        m2_all = spool.tile([128, NRT], dt)
        nc.vector.scalar_tensor_tensor(m2_all[:], mu_all[:], -1.0, rstd_all[:],
                                       op0=OP.mult, op1=OP.mult)
        # B4c: normalize + ln scale/shift
        for rt in range(NRT):
            a_n = an_all[:, rt, :]
            q = work2.tile([128, DM], dt, tag="q")
            nc.vector.tensor_scalar(q[:], a_n, rstd_all[:, rt:rt + 1],
                                    m2_all[:, rt:rt + 1],
                                    op0=OP.mult, op1=OP.add)
            ln = work2.tile([128, DM], dt, tag="ln")
            nc.vector.tensor_mul(ln[:], q[:], lnw_bc[:])
            nc.vector.tensor_add(an_all[:, rt, :], ln[:], lnb_bc[:])

        # B5: gelu + residual + transpose -> hbT (gelu table)
        last_gelu = None
        for rt in range(NRT):
            cs = rt * 128
            g = work2.tile([128, DM], dt, tag="g")
            i_g = nc.scalar.activation(g[:], an_all[:, rt, :], AF.Gelu)
            if rt == 0:
                add_dep_helper(i_g.ins, last_b4_act.ins, sync=True,
                               reason="act table: gelu after nle")
            last_gelu = i_g
            hb_row = work2.tile([128, DM], dt, tag="hb_row")
            nc.vector.tensor_add(hb_row[:], g[:], hrow_all[:, rt, :])
            ps_tb = mm_tile([DM, 128])
            nc.tensor.transpose(ps_tb[:], hb_row[:], ident[:])
            nc.scalar.copy(hbT[:, cs:cs + 128], ps_tb[:])

        if DEBUG:
            dma(dbg["d_hT"], hT[:])
            dma(dbg["d_hbT"], hbT[:])

        # ---- stage D: in_proj -> xcT (padded); z -> silu_z (padded) ----
        xcT = [big.tile([128, LT], dt, tag=f"xcT{cc}", name=f"xcT{cc}") for cc in range(2)]
        sluz = [big.tile([128, NROW], dt, tag=f"sluz{cc}", name=f"sluz{cc}") for cc in range(2)]
        for cc in range(2):
            # only the gap columns need zeroing (conv taps read them)
            nc.gpsimd.memset(
                xcT[cc][:].rearrange("p (b l) -> p b l", b=RPC)[:, :, 0:GAP], 0.0)
        ccw = [(i * 512, min(512, NROW - i * 512))
               for i in range((NROW + 511) // 512)]
        first_silu = None
        for pc in range(4):
            cchunk, isx = (pc % 2), (pc < 2)
            for (c0, cw) in ccw:
                nblk_c = cw // PN
                ps_x = mm_tile([128, 512])
                nc.tensor.matmul(ps_x[:, :cw],
                                 w["in_proj_wT"][:, pc * 128:(pc + 1) * 128],
                                 hbT[:, c0:c0 + cw], start=True, stop=True)
                if isx:
                    p0 = (c0 // PN) * LP
                    dview = xcT[cchunk][:, p0:p0 + nblk_c * LP].rearrange(
                        "p (b l) -> p b l", b=nblk_c)[:, :, GAP:LP]
                    sview = ps_x[:, :cw].rearrange("p (b l) -> p b l", b=nblk_c)
                    nc.vector.tensor_copy(dview, sview)
                else:
                    i_s = nc.scalar.activation(sluz[cchunk][:, c0:c0 + cw],
                                               ps_x[:, :cw], AF.Silu)
                    if first_silu is None:
                        first_silu = i_s
                        add_dep_helper(i_s.ins, last_gelu.ins, sync=True,
                                       reason="act table: silu after gelu")

        # ---- stage E: causal depthwise conv + silu (chunked, no in-place) ----
        xc2T = [big.tile([128, LT], dt, tag=f"xc2T{cc}", name=f"xc2T{cc}")
                for cc in range(2)]
        for cc in range(2):
            nc.gpsimd.memset(
                xc2T[cc][:].rearrange("p (b l) -> p b l", b=RPC)[:, :, 0:GAP],
                0.0)
            wsl = w["conv_w2"][cc]
            for si in range(NSC):
                c0 = si * SCW
                cw_ = SCW - GAP
                t1c = scanp.tile([128, cw_], dt, tag="a_t", name="cv1")
                nc.vector.tensor_scalar(t1c[:], xcT[cc][:, c0:c0 + cw_],
                                        wsl[:, 0:1], None, op0=OP.mult)
                t2c = scanp.tile([128, cw_], dt, tag="b_t", name="cv2")
                nc.vector.scalar_tensor_tensor(t2c[:],
                                               xcT[cc][:, c0 + 1:c0 + 1 + cw_],
                                               wsl[:, 1:2], t1c[:],
                                               op0=OP.mult, op1=OP.add)
                t3c = scanp.tile([128, cw_], dt, tag="a_t", name="t3c")
                nc.vector.scalar_tensor_tensor(t3c[:],
                                               xcT[cc][:, c0 + 2:c0 + 2 + cw_],
                                               wsl[:, 2:3], t2c[:],
                                               op0=OP.mult, op1=OP.add)
                t4c = scanp.tile([128, cw_], dt, tag="b_t", name="t4c")
                nc.vector.scalar_tensor_tensor(t4c[:],
                                               xcT[cc][:, c0 + 3:c0 + 3 + cw_],
                                               wsl[:, 3:4], t3c[:],
                                               op0=OP.mult, op1=OP.add)
                i_cs = nc.scalar.activation(xc2T[cc][:, c0 + GAP:c0 + SCW],
                                            t4c[:], AF.Silu,
                                            bias=w["conv_b_col"][cc][:])
                last_silu = i_cs
        if DEBUG:
            for cc in range(2):
                dma(dbg["d_xc2T"][cc * 128:(cc + 1) * 128, :], xc2T[cc][:])

        # ---- stage F: x_proj -> (dt,Bm,Cm); dt_proj -> delta; du ----
        # separate tiles so each starts at partition 0 (matmul base rule)
        dblD = big.tile([DTR, LT], dt, tag="hT")  # reuse hT slot (dead)
        dblB_t = big.tile([DS, LT], mybir.dt.bfloat16, tag="dblB")
        dblC_t = big.tile([DS, LT], mybir.dt.bfloat16, tag="dblC")
        dblB = dblB_t[:]
        dblC = dblC_t[:]
        for si in range(NSC):
            c0 = si * SCW
            for (lo, hi, dst) in [(0, DTR, dblD[:]), (DTR, DTR + DS, dblB),
                                  (DTR + DS, DTR + 2 * DS, dblC)]:
                ps_d = mm_tile([hi - lo, SCW])
                for cc in range(2):
                    nc.tensor.matmul(ps_d[:],
                                     w["x_proj_wT"][cc][:, lo:hi],
                                     xc2T[cc][:, c0:c0 + SCW],
                                     start=(cc == 0), stop=(cc == 1))
                nc.scalar.copy(dst[:, c0:c0 + SCW], ps_d[:])
        if DEBUG:
            dma(dbg["d_dblT"][0:DTR, :], dblD[:])
            dma(dbg["d_dblB"], dblB)
            dma(dbg["d_dblC"], dblC)

        deltaT = [big.tile([128, LT], dt, tag=f"xcT{cc}", name=f"deltaT{cc}") for cc in range(2)]
        duT = [big.tile([128, LT], dt, tag=t, name=f"duT_{t}") for t in ("convacc", "hbT")]
        # Exp pass (staged into duT), then Ln pass -> softplus, grouped so the
        # act-table switches once per function
        for cc in range(2):
            for si in range(NSC):
                c0 = si * SCW
                ps_dt = mm_tile([128, SCW])
                nc.tensor.matmul(ps_dt[:],
                                 w["dt_proj_wT"][:, cc * 128:(cc + 1) * 128],
                                 dblD[:][:, c0:c0 + SCW], start=True, stop=True)
                i_e1 = nc.scalar.activation(duT[cc][:, c0:c0 + SCW], ps_dt[:],
                                            AF.Exp,
                                            bias=w["dt_proj_b_col"][cc][:])
                if cc == 0 and si == 0:
                    add_dep_helper(i_e1.ins, last_silu.ins, sync=True,
                                   reason="act table: exp after silu")
                last_exp_f = i_e1
        first_agen = None
        last_softplus = None
        for cc in range(2):
            for si in range(NSC):
                c0 = si * SCW
                i_ln = nc.scalar.activation(deltaT[cc][:, c0:c0 + SCW],
                                            duT[cc][:, c0:c0 + SCW],
                                            AF.Ln, bias=1.0)
                add_dep_helper(i_ln.ins, last_exp_f.ins, sync=True,
                               reason="act table: ln after exp pass")
                last_softplus = i_ln
                dv = lambda t: t[:, c0:c0 + SCW].rearrange(
                    "p (b l) -> p b l", b=SCB)
                nc.gpsimd.memset(dv(duT[cc])[:, :, 0:GAP], 0.0)
                nc.vector.tensor_mul(dv(duT[cc])[:, :, GAP:LP],
                                     dv(deltaT[cc])[:, :, GAP:LP],
                                     dv(xc2T[cc])[:, :, GAP:LP])
                # poison delta gaps so exp(A*delta)=0 there (state reset)
                nc.vector.memset(dv(deltaT[cc])[:, :, 0:GAP], POISON)
        if DEBUG:
            for cc in range(2):
                dma(dbg["d_deltaT"][cc * 128:(cc + 1) * 128, :], deltaT[cc][:])
                dma(dbg["d_duT"][cc * 128:(cc + 1) * 128, :], duT[cc][:])

        # ---- stage G: selective scan ----
        # one-hot row-selection matrix: sel[i, d*128+m] = (i == d)
        sel = cpool.tile([DS, DS * 128], mybir.dt.bfloat16)
        nc.gpsimd.memset(sel[:], 0.0)
        nc.gpsimd.affine_select(out=sel[:], in_=sel[:],
                                compare_op=OP.not_equal, fill=1.0,
                                base=0, pattern=[[-1, DS], [0, 128]],
                                channel_multiplier=1)
        y2T = [big.tile([128, NROW], dt, tag=f"y2T{cc}", name=f"y2T{cc}") for cc in range(2)]

        for si in range(NSC):
            c0 = si * SCW
            ps_ys = [ps_y.tile([128, SCW], dt, tag=f"ps_y{cc}", name=f"ps_ys{cc}")
                     for cc in range(2)]
            for d in range(DS):
                ps_bm = ps_bc.tile([128, SCW], dt, tag="ps_bm")
                nc.tensor.matmul(ps_bm[:],
                                 sel[:, d * 128:(d + 1) * 128],
                                 dblB[:, c0:c0 + SCW],
                                 start=True, stop=True)
                ps_cm = ps_bc.tile([128, SCW], dt, tag="ps_cm")
                nc.tensor.matmul(ps_cm[:],
                                 sel[:, d * 128:(d + 1) * 128],
                                 dblC[:, c0:c0 + SCW],
                                 start=True, stop=True)
                for cc in range(2):
                    a_t = scanp.tile([128, SCW], mybir.dt.bfloat16, tag="sc_a",
                                     name="a_t")
                    i_ag = nc.scalar.activation(a_t[:],
                                                deltaT[cc][:, c0:c0 + SCW],
                                                AF.Exp,
                                                scale=A_sb[cc][:, d:d + 1])
                    if first_agen is None:
                        first_agen = i_ag
                        add_dep_helper(i_ag.ins, last_softplus.ins, sync=True,
                                       reason="act table: exp after ln")
                    b_t = scanp.tile([128, SCW], mybir.dt.bfloat16, tag="sc_b",
                                     name="b_t")
                    nc.vector.tensor_mul(b_t[:], duT[cc][:, c0:c0 + SCW],
                                         ps_bm[:])
                    h_t = scanp.tile([128, SCW], mybir.dt.bfloat16, tag="h_t")
                    nc.vector.tensor_tensor_scan(
                        h_t[:], a_t[:], b_t[:], initial=0.0,
                        op0=OP.mult, op1=OP.add)
                    p_t = scanp.tile([128, SCW], mybir.dt.bfloat16, tag="p_t")
                    nc.vector.tensor_mul(p_t[:], h_t[:], ps_cm[:])
                    nc.tensor.matmul(ps_ys[cc][:], identb[:], p_t[:],
                                     start=(d == 0), stop=(d == DS - 1))
            d0 = si * SCB * PN
            for cc in range(2):
                t1s = scanp.tile([128, SCW], dt, tag="t1s")
                nc.vector.scalar_tensor_tensor(
                    t1s[:], xc2T[cc][:, c0:c0 + SCW],
                    w["D_col"][cc][:], ps_ys[cc][:],
                    op0=OP.mult, op1=OP.add)
                nc.vector.tensor_mul(
                    y2T[cc][:, d0:d0 + SCB * PN].rearrange(
                        "p (b l) -> p b l", b=SCB),
                    t1s[:].rearrange("p (b l) -> p b l", b=SCB)[:, :, GAP:LP],
                    sluz[cc][:, d0:d0 + SCB * PN].rearrange(
                        "p (b l) -> p b l", b=SCB))
        if DEBUG:
            for cc in range(2):
                dma(dbg["d_y2T"][cc * 128:(cc + 1) * 128, :], y2T[cc][:])

        # ---- stage H: out_proj (compact, bf16 out) ----
        moT = big.tile([DM, NROW], mybir.dt.bfloat16, tag="sluz0", name="moT")
        CW = SCB * PN
        for si in range(NSC):
            d0 = si * CW
            ps_mo = mm_tile([DM, CW])
            for cc in range(2):
                nc.tensor.matmul(ps_mo[:],
                                 w["out_proj_wT"][cc][:],
                                 y2T[cc][:, d0:d0 + CW],
                                 start=(cc == 0), stop=(cc == 1))
            nc.scalar.copy(moT[:, d0:d0 + CW], ps_mo[:])
        if DEBUG:
            dma(dbg["d_moT"], moT[:])
        scanp_cm.__exit__(None, None, None)

        # ---- stage I: mlp2 (gelu) + mlp3 + denorm + output ----
        w2pool = P(name="w2p", bufs=1)
        w2sb = w2pool.tile([DM, PN * 2 * PRED], mybir.dt.bfloat16)
        dma(w2sb[:], IN["w2s"])
        w2v = w2sb[:].rearrange("p (n j) -> p n j", n=PN)
        ps_o2 = ps_y.tile([128, RPC], dt, tag="ps_y0")
        ps_o2b = ps_y.tile([2 * PRED - 128, RPC], dt, tag="ps_y1")
        mo_v = moT[:].rearrange("p (b l) -> p b l", b=RPC)
        for pn in range(PN):
            rhs = mo_v[:, :, pn:pn + 1]
            nc.tensor.matmul(ps_o2[:], w2v[:, pn, 0:128], rhs,
                             start=(pn == 0), stop=(pn == PN - 1))
            nc.tensor.matmul(ps_o2b[:], w2v[:, pn, 128:2 * PRED], rhs,
                             start=(pn == 0), stop=(pn == PN - 1))
        o2a = work.tile([128, RPC], dt, tag="o2a")
        nc.scalar.activation(o2a[:], ps_o2[:], AF.Gelu,
                             bias=w["mlp2_b_col"][0][:])
        o2b = work.tile([2 * PRED - 128, RPC], dt, tag="o2b")
        nc.scalar.activation(o2b[:], ps_o2b[:], AF.Gelu,
                             bias=w["mlp2_b_col"][1][:])
        ps_o3 = mm_tile([PRED, RPC])
        nc.tensor.matmul(ps_o3[:], w["mlp3_wT"][0][:], o2a[:],
                         start=True, stop=False)
        nc.tensor.matmul(ps_o3[:], w["mlp3_wT"][1][:], o2b[:],
                         start=False, stop=True)
        o3T = work.tile([PRED, RPC], dt, tag="o3T")
        nc.vector.tensor_copy(o3T[:], ps_o3[:])
        ps_o3t = mm_tile([RPC, PRED])
        nc.tensor.transpose(ps_o3t[:], o3T[:], ident[0:PRED, 0:PRED])

        b3row = work.tile([1, PRED], dt, tag="b3row")
        dma(b3row[:], IN["mlp3_b_row"])
        ps_b3 = mm_tile([RPC, PRED])
        nc.tensor.matmul(ps_b3[:], ones1[:, 0:RPC], b3row[:],
                         start=True, stop=True)
        den = work.tile([RPC, PRED], dt, tag="den")
        nc.vector.tensor_scalar(den[:], ps_b3[:], t_den[:], u_den[:],
                                op0=OP.mult, op1=OP.add)
        out_sb = work.tile([RPC, PRED], dt, tag="out_sb")
        nc.vector.scalar_tensor_tensor(out_sb[:], ps_o3t[:], t_den[:], den[:],
                                       op0=OP.mult, op1=OP.add)
        dma(out_d, out_sb[:])

    nc.compile()
    _cache[key] = nc
    return nc


def kernel(**inputs):
    nc = build_program()
    in_maps = prep_inputs(inputs)
    res = run_bass_kernel_spmd(nc, in_maps, list(range(NCORES)))
    return assemble(res.results)


if __name__ == "__main__":
    import reference as R
    inp = R.setup_inputs()
    out = kernel(**{k: np.asarray(v) for k, v in inp.items()})
    print("kernel out", out.shape, out.dtype, np.abs(out).max())



# revision 46
# speedup vs baseline: 10.7958x; 1.9222x over previous
"""Trainium2 Bass kernel for nn_Backbone1_62947040690721.

Data-parallel over the fused B*NV block axis: 336 independent per-series
problems, 42 per NeuronCore across 8 cores. All weights replicated.

Layouts (per core, 42 blocks):
  - row layout: (block,patch) rows on partitions (128 rows = 2 blocks/tile)
  - T layout:   features on partitions, l = concat of blocks on the free axis,
                padded to 67 cols/block (3 zero "gap" cols + 64 data cols).
                The gaps give causal-conv zero padding and selective-scan
                state resets between blocks for free.
The selective scan runs as hardware tensor_tensor_scan instructions, one per
(state dim d, channel chunk), scanning 7 blocks' timelines per call; scan
chunks are block-aligned so every chunk self-resets at its leading gap cols.
"""

import sys

sys.path.insert(0, "/opt/trn_rl_repo")

import numpy as np

import concourse.bass as bass
import concourse.mybir as mybir
import concourse.tile as tile
from concourse import bacc
from concourse.bass_utils import run_bass_kernel_spmd

F32 = mybir.dt.float32
F32R = mybir.dt.float32r
AF = mybir.ActivationFunctionType
OP = mybir.AluOpType
AX = mybir.AxisListType

# model dims
B, T, NV = 16, 512, 21
PS, STRIDE, PRED = 16, 8, 96
DM, DS, DC = 128, 16, 4
DIN = 2 * DM          # 256
DTR = 8
S_EA = 512
PN = (T - PS) // STRIDE + 1 + 1  # 64
EPS = 1e-5

NCORES = 8
NBLK = B * NV          # 336
RPC = NBLK // NCORES   # 42 blocks per core
NROW = RPC * PN        # 2688 compact rows per core
NRT = NROW // 128      # 21 row tiles
GAP = 3                # zero-pad cols before each block
LP = PN + GAP          # 67 padded cols per block
LT = RPC * LP          # 2814 padded timeline length
SCB = 14               # blocks per scan chunk
SCW = SCB * LP         # 938 scan chunk width (>=256 keeps fp32r full rate)
NSC = RPC // SCB       # 3 scan chunks
POISON = 1.0e30

DEBUG = False          # set True (before first kernel build) for stage taps
REPEAT = 1             # build the body N times (for differential timing)

_cache = {}


def _r(x):
    return np.ascontiguousarray(np.asarray(x, dtype=np.float32))


# packed-weight layouts: (name, rows, cols); offsets assigned in order
PACKF_SPEC = [
    ("mlp1_wT", PS, DM),
    ("mk_wT", DM, S_EA),
    ("in_proj_wT", DM, 2 * DIN),
    ("conv_w2_0", 128, DC), ("conv_w2_1", 128, DC),
    ("conv_b_col_0", 128, 1), ("conv_b_col_1", 128, 1),
    ("dt_proj_b_col_0", 128, 1), ("dt_proj_b_col_1", 128, 1),
    ("D_col_0", 128, 1), ("D_col_1", 128, 1),
    ("A_0", 128, DS), ("A_1", 128, DS),
    ("mlp2_b_col_0", 128, 1), ("mlp2_b_col_1", 2 * PRED - 128, 1),
    ("mlp3_wT_0", 128, PRED), ("mlp3_wT_1", 2 * PRED - 128, PRED),
    ("b1_bc", 128, DM), ("lnw_bc", 128, DM), ("lnb_bc", 128, DM),
    ("w1s_bc", 128, DM),
    ("b3_bc", RPC, PRED),
]
PACKB_SPEC = [
    ("x_proj_wT_0", 128, DTR + 2 * DS), ("x_proj_wT_1", 128, DTR + 2 * DS),
    ("out_proj_wT_0", 128, DM), ("out_proj_wT_1", 128, DM),
    ("dt_proj_wT", DTR, DIN),
    ("mv_aug", 128, 4 * (DM + 1)),
]


def _pack_layout(spec):
    off, out = 0, {}
    for name, rows, cols in spec:
        out[name] = (rows, off, off + cols)
        off += cols
    return out, off


PACKF_LAYOUT, NPF = _pack_layout(PACKF_SPEC)
PACKB_LAYOUT, NPB = _pack_layout(PACKB_SPEC)


def prep_inputs(inputs):
    """Full inputs -> per-core input maps (pure data movement on host)."""
    x = _r(inputs["x"])
    xbn = np.ascontiguousarray(x.transpose(0, 2, 1).reshape(NBLK, T))
    xp = np.concatenate([xbn, np.repeat(xbn[:, -1:], STRIDE, axis=1)], axis=1)
    idx = np.arange(PN)[:, None] * STRIDE + np.arange(PS)[None, :]
    pat = xp[:, idx]                                     # (336, 64, 16)
    patT = np.ascontiguousarray(pat.transpose(2, 0, 1))  # (16, 336, 64)
    wv = np.tile(_r(inputs["revin_w"]), B).reshape(NBLK, 1)
    bv = np.tile(_r(inputs["revin_b"]), B).reshape(NBLK, 1)

    import ml_dtypes
    bf16 = ml_dtypes.bfloat16
    mlp2_wT = _r(inputs["mlp2_w"]).T       # (8192, 192)
    w2s = np.ascontiguousarray(
        mlp2_wT.reshape(PN, DM, 2 * PRED).transpose(1, 0, 2)
    ).astype(bf16)  # (128, 64, 192) bf16

    mlp1_wT = _r(inputs["mlp1_w"]).T                     # (16,128)
    A = -np.exp(_r(inputs["A_log"]))                     # (256,16)
    conv_w2 = _r(inputs["conv_w"])[:, 0, :]              # (256,4)
    conv_b = _r(inputs["conv_b"]).reshape(DIN, 1)
    dt_b = _r(inputs["dt_proj_b"]).reshape(DIN, 1)
    D_col = _r(inputs["D_ssm"]).reshape(DIN, 1)
    mlp2_b = _r(inputs["mlp2_b"]).reshape(2 * PRED, 1)
    mlp3_wT = _r(inputs["mlp3_w"]).T                     # (192,96)
    fvals = {
        "mlp1_wT": mlp1_wT,
        "mk_wT": _r(inputs["mk_w"]).T,
        "in_proj_wT": _r(inputs["in_proj_w"]).T,
        "conv_w2_0": conv_w2[:128], "conv_w2_1": conv_w2[128:],
        "conv_b_col_0": conv_b[:128], "conv_b_col_1": conv_b[128:],
        "dt_proj_b_col_0": dt_b[:128], "dt_proj_b_col_1": dt_b[128:],
        "D_col_0": D_col[:128], "D_col_1": D_col[128:],
        "A_0": A[:128], "A_1": A[128:],
        "mlp2_b_col_0": mlp2_b[:128], "mlp2_b_col_1": mlp2_b[128:],
        "mlp3_wT_0": mlp3_wT[:128], "mlp3_wT_1": mlp3_wT[128:],
        "b1_bc": np.broadcast_to(_r(inputs["mlp1_b"])[None, :], (128, DM)),
        "lnw_bc": np.broadcast_to(_r(inputs["ln_w"])[None, :], (128, DM)),
        "lnb_bc": np.broadcast_to(_r(inputs["ln_b"])[None, :], (128, DM)),
        "w1s_bc": np.broadcast_to(mlp1_wT.sum(0)[None, :], (128, DM)),
        "b3_bc": np.broadcast_to(_r(inputs["mlp3_b"])[None, :], (RPC, PRED)),
    }
    packf = np.zeros((128, NPF), np.float32)
    for name, (rows, c0, c1) in PACKF_LAYOUT.items():
        packf[:rows, c0:c1] = fvals[name]

    mv_wT = _r(inputs["mv_w"]).T                         # (512,128)
    mv_aug = np.ones((128, 4 * (DM + 1)), np.float32)
    for sc in range(4):
        mv_aug[:, sc * 129:sc * 129 + DM] = mv_wT[sc * 128:(sc + 1) * 128]
    x_proj_wT = _r(inputs["x_proj_w"]).T                 # (256,40)
    out_proj_wT = _r(inputs["out_proj_w"]).T             # (256,128)
    bvals = {
        "x_proj_wT_0": x_proj_wT[:128], "x_proj_wT_1": x_proj_wT[128:],
        "out_proj_wT_0": out_proj_wT[:128], "out_proj_wT_1": out_proj_wT[128:],
        "dt_proj_wT": _r(inputs["dt_proj_w"]).T,
        "mv_aug": mv_aug,
    }
    packb = np.zeros((128, NPB), bf16)
    for name, (rows, c0, c1) in PACKB_LAYOUT.items():
        packb[:rows, c0:c1] = bvals[name].astype(bf16)

    shared = {"packf": packf, "packb": packb, "w2s": w2s}
    in_maps = []
    for c in range(NCORES):
        lo, hi = c * RPC, (c + 1) * RPC
        m = dict(shared)
        m["xrow"] = np.ascontiguousarray(xbn[lo:hi])                  # (42,512)
        m["patT"] = np.ascontiguousarray(patT[:, lo:hi, :]).reshape(PS, NROW)
        m["wv"] = np.ascontiguousarray(wv[lo:hi])
        m["bv"] = np.ascontiguousarray(bv[lo:hi])
        in_maps.append(m)
    return in_maps


def assemble(results):
    outs = np.concatenate([r["out"] for r in results], axis=0)  # (336, 96)
    out = outs.reshape(B, NV, PRED).transpose(0, 2, 1)
    return np.ascontiguousarray(out.astype(np.float32))


# ---------------------------------------------------------------------------
# program builder
# ---------------------------------------------------------------------------

def _decl_inputs(nc):
    d = {}
    spec = {
        "xrow": (RPC, T), "patT": (PS, NROW), "wv": (RPC, 1), "bv": (RPC, 1),
        "packf": (128, NPF), "packb": (128, NPB),
        "w2s": (DM, PN, 2 * PRED),
    }
    bf16_ins = ("w2s", "packb")
    for name, shape in spec.items():
        dty = mybir.dt.bfloat16 if name in bf16_ins else F32
        d[name] = nc.dram_tensor(name, list(shape), dty,
                                 kind="ExternalInput").ap()
    return d


def build_program():
    key = ("nc", REPEAT, DEBUG)
    if key in _cache:
        return _cache[key]
    nc = bacc.Bacc("TRN2", target_bir_lowering=False, debug=False,
                   num_devices=NCORES)
    IN = _decl_inputs(nc)
    out_d = nc.dram_tensor("out", [RPC, PRED], F32, kind="ExternalOutput").ap()

    dbg = {}
    if DEBUG:
        for name, shape in [
            ("d_hT", (DM, NROW)), ("d_hbT", (DM, NROW)),
            ("d_xc2T", (DIN, LT)), ("d_deltaT", (DIN, LT)),
            ("d_duT", (DIN, LT)), ("d_y2T", (DIN, NROW)),
            ("d_moT", (DM, NROW)), ("d_dblT", (DTR + 2 * DS, LT)),
            ("d_dblB", (DS, LT)), ("d_dblC", (DS, LT)),
            ("d_stats", (RPC, 6)),
        ]:
            dty = (mybir.dt.bfloat16
                   if name in ("d_dblB", "d_dblC", "d_moT") else F32)
            dbg[name] = nc.dram_tensor(name, list(shape), dty,
                                       kind="ExternalOutput").ap()

    from contextlib import ExitStack
    from concourse.masks import make_identity
    from concourse.tile import add_dep_helper

    with tile.TileContext(nc) as tc:
      for _rep in range(REPEAT):
       with ExitStack() as ctx:
        P = lambda **kw: ctx.enter_context(tc.tile_pool(**kw))
        wpool = P(name="weights", bufs=1)
        cpool = P(name="consts", bufs=1)
        spool = P(name="statp", bufs=1)
        big = P(name="bigact", bufs=1)
        work = P(name="work", bufs=2)
        work2 = P(name="work2", bufs=2)
        scanp_cm = tc.tile_pool(name="scanp", bufs=3)
        scanp = scanp_cm.__enter__()
        # PSUM: mm(2 bufs) + y = 8 banks max
        ps_mm = P(name="ps_mm", bufs=2, space="PSUM")
        ps_y = P(name="ps_y", bufs=1, space="PSUM")

        dt = F32
        dtb = mybir.dt.bfloat16

        def dma(dst, src):
            nc.sync.dma_start(out=dst, in_=src)

        def mm_tile(shape, tag="mm"):
            return ps_mm.tile(list(shape), dt, tag=tag, name=tag)

        # ---- constants ----
        ident = cpool.tile([128, 128], dt)
        make_identity(nc, ident[:])
        identb = cpool.tile([128, 128], mybir.dt.bfloat16)
        make_identity(nc, identb[:])
        epsc = cpool.tile([128, 1], dt)
        nc.vector.memset(epsc[:], EPS)

        # input data first so stage A starts immediately
        xr = big.tile([RPC, T], dt, tag="xrow")
        dma(xr[:], IN["xrow"])
        patT_sb = big.tile([PS, NROW], dt, tag="patT")
        dma(patT_sb[:], IN["patT"])
        wv = spool.tile([RPC, 1], dt)
        dma(wv[:], IN["wv"])
        bv = spool.tile([RPC, 1], dt)
        dma(bv[:], IN["bv"])

        # packed weights: one DMA per dtype (HWDGE slots are ~625ns each)
        packf_t = wpool.tile([128, NPF], dt)
        dma(packf_t[:], IN["packf"])
        packb_t = wpool.tile([128, NPB], dtb)
        dma(packb_t[:], IN["packb"])

        def WF(name):
            rows, c0, c1 = PACKF_LAYOUT[name]
            return packf_t[0:rows, c0:c1]

        def WB(name):
            rows, c0, c1 = PACKB_LAYOUT[name]
            return packb_t[0:rows, c0:c1]

        w = {
            "mlp1_wT": WF("mlp1_wT"), "mk_wT": WF("mk_wT"),
            "in_proj_wT": WF("in_proj_wT"),
            "dt_proj_wT": WB("dt_proj_wT"),
            "conv_w2": [WF("conv_w2_0"), WF("conv_w2_1")],
            "conv_b_col": [WF("conv_b_col_0"), WF("conv_b_col_1")],
            "dt_proj_b_col": [WF("dt_proj_b_col_0"), WF("dt_proj_b_col_1")],
            "D_col": [WF("D_col_0"), WF("D_col_1")],
            "x_proj_wT": [WB("x_proj_wT_0"), WB("x_proj_wT_1")],
            "out_proj_wT": [WB("out_proj_wT_0"), WB("out_proj_wT_1")],
            "mlp2_b_col": [WF("mlp2_b_col_0"), WF("mlp2_b_col_1")],
            "mlp3_wT": [WF("mlp3_wT_0"), WF("mlp3_wT_1")],
        }
        mv_aug = WB("mv_aug")
        A_sb = [WF("A_0"), WF("A_1")]
        b1_bc = WF("b1_bc")
        lnw_bc = WF("lnw_bc")
        lnb_bc = WF("lnb_bc")
        w1s_bc = WF("w1s_bc")
        b3_bc = WF("b3_bc")

        # ---- stage A: RevIN stats ----
        sumx = spool.tile([RPC, 1], dt)
        nc.vector.reduce_sum(sumx[:], xr[:], axis=AX.X)
        mean = spool.tile([RPC, 1], dt)
        nc.vector.tensor_scalar_mul(mean[:], sumx[:], 1.0 / T)
        sq = work.tile([RPC, T], dt, tag="sq", bufs=1)
        sumx2 = spool.tile([RPC, 1], dt)
        nc.scalar.activation(sq[:], xr[:], AF.Square, accum_out=sumx2[:])
        ex2 = spool.tile([RPC, 1], dt)
        nc.vector.tensor_scalar_mul(ex2[:], sumx2[:], 1.0 / T)
        msq = spool.tile([RPC, 1], dt)
        nc.vector.tensor_mul(msq[:], mean[:], mean[:])
        var = spool.tile([RPC, 1], dt)
        nc.vector.tensor_sub(var[:], ex2[:], msq[:])
        lnv = spool.tile([RPC, 1], dt)
        nc.scalar.activation(lnv[:], var[:], AF.Ln, bias=epsc[0:RPC, :])
        std = spool.tile([RPC, 1], dt)
        nc.scalar.activation(std[:], lnv[:], AF.Exp, scale=0.5)
        istd = spool.tile([RPC, 1], dt)
        nc.scalar.activation(istd[:], lnv[:], AF.Exp, scale=-0.5)

        s_n = spool.tile([RPC, 1], dt)
        nc.vector.tensor_mul(s_n[:], wv[:], istd[:])
        o_n0 = spool.tile([RPC, 1], dt)
        nc.vector.scalar_tensor_tensor(o_n0[:], mean[:], -1.0, s_n[:],
                                       op0=OP.mult, op1=OP.mult)
        o_n = spool.tile([RPC, 1], dt)
        nc.vector.tensor_add(o_n[:], o_n0[:], bv[:])

        wq = spool.tile([RPC, 1], dt)
        nc.vector.tensor_scalar_add(wq[:], wv[:], EPS * EPS)
        rw = spool.tile([RPC, 1], dt)
        nc.vector.reciprocal(rw[:], wq[:])
        t_den = spool.tile([RPC, 1], dt)
        nc.vector.tensor_mul(t_den[:], std[:], rw[:])
        u_den0 = spool.tile([RPC, 1], dt)
        nc.vector.scalar_tensor_tensor(u_den0[:], bv[:], -1.0, t_den[:],
                                       op0=OP.mult, op1=OP.mult)
        u_den = spool.tile([RPC, 1], dt)
        nc.vector.tensor_add(u_den[:], u_den0[:], mean[:])

        svec = spool.tile([RPC, 2], dt)
        nc.vector.tensor_copy(svec[:, 0:1], s_n[:])
        nc.vector.tensor_copy(svec[:, 1:2], o_n[:])
        if DEBUG:
            stats = spool.tile([RPC, 6], dt)
            for i, tl in enumerate([mean, std, s_n, o_n, t_den, u_den]):
                nc.vector.tensor_copy(stats[:, i:i + 1], tl[:])
            dma(dbg["d_stats"], stats[:])

        # ---- stage B: mlp1 + external attention + LN + gelu + residual ----
        # structured as function-grouped passes to avoid ACT table thrash
        hT = big.tile([DM, NROW], dt, tag="hT")
        hbT = big.tile([DM, NROW], dt, tag="hbT")
        hrow_all = big.tile([128, NRT, DM], dt, tag="sluz0")
        an_all = big.tile([128, NRT, DM], dt, tag="sluz1")
        # exp(logits), bf16, two halves (sc 0-1 / 2-3), each [128, NRT, 256]
        exp4 = [big.tile([128, NRT, 2 * 128], dtb, tag=tg, name=f"exp4_{i}")
                for i, tg in enumerate(["xcT0", "xcT1"])]

        # B1: mlp1 + revin fold + transpose -> hT, hrow_all
        for rt in range(NRT):
            cs = rt * 128
            so_row = work.tile([128, 2], dt, tag="so_row")
            dma(so_row[:],
                svec[rt * 2:rt * 2 + 2, :].unsqueeze(1).broadcast_to((2, PN, 2)))
            ps_h = mm_tile([128, DM])
            nc.tensor.matmul(ps_h[:], patT_sb[:, cs:cs + 128], w["mlp1_wT"],
                             start=True, stop=True)
            t1 = work.tile([128, DM], dt, tag="t1")
            nc.vector.scalar_tensor_tensor(t1[:], w1s_bc[:], so_row[:, 1:2],
                                           b1_bc[:], op0=OP.mult, op1=OP.add)
            nc.vector.scalar_tensor_tensor(hrow_all[:, rt, :], ps_h[:],
                                           so_row[:, 0:1], t1[:],
                                           op0=OP.mult, op1=OP.add)
            ps_tr = mm_tile([DM, 128])
            nc.tensor.transpose(ps_tr[:], hrow_all[:, rt, :], ident[:])
            nc.scalar.copy(hT[:, cs:cs + 128], ps_tr[:])

        # B2: logits + exp (exp table), batched 2 key-chunks per ACT op
        for rt in range(NRT):
            cs = rt * 128
            for half in range(2):
                ps_l = mm_tile([128, 256])
                for j in range(2):
                    sc = half * 2 + j
                    nc.tensor.matmul(ps_l[:, j * 128:(j + 1) * 128],
                                     w["mk_wT"][:, sc * 128:(sc + 1) * 128],
                                     hT[:, cs:cs + 128], start=True, stop=True)
                nc.scalar.activation(exp4[half][:, rt, :], ps_l[:], AF.Exp)

        # B3: attnv (+sum column) + normalize
        for rt in range(NRT):
            ps_at = ps_y.tile([128, DM + 1], dt, tag="ps_y0", name="ps_at")
            for sc in range(4):
                nc.tensor.matmul(ps_at[:],
                                 exp4[sc // 2][:, rt,
                                               (sc % 2) * 128:(sc % 2 + 1) * 128],
                                 mv_aug[:, sc * 129:(sc + 1) * 129],
                                 start=(sc == 0), stop=(sc == 3))
            rec = work.tile([128, 1], dt, tag="rec")
            nc.vector.reciprocal(rec[:], ps_at[:, DM:DM + 1])
            nc.vector.tensor_scalar_mul(an_all[:, rt, :], ps_at[:, 0:DM], rec[:])

        # B4a: LN stats for all row tiles (Square is in every act table)
        mu_all = spool.tile([128, NRT], dt)
        varr_all = spool.tile([128, NRT], dt)
        for rt in range(NRT):
            a_n = an_all[:, rt, :]
            sm = work.tile([128, 1], dt, tag="sm")
            nc.vector.reduce_sum(sm[:], a_n, axis=AX.X)
            nc.vector.tensor_scalar_mul(mu_all[:, rt:rt + 1], sm[:], 1.0 / DM)
            sqs = work2.tile([128, DM], dt, tag="sqs")
            ssq = work.tile([128, 1], dt, tag="ssq")
            nc.scalar.activation(sqs[:], a_n, AF.Square, accum_out=ssq[:])
            ex2r = work.tile([128, 1], dt, tag="ex2r")
            nc.vector.tensor_scalar_mul(ex2r[:], ssq[:], 1.0 / DM)
            msqr = work.tile([128, 1], dt, tag="msqr")
            nc.vector.tensor_mul(msqr[:], mu_all[:, rt:rt + 1],
                                 mu_all[:, rt:rt + 1])
            nc.vector.tensor_sub(varr_all[:, rt:rt + 1], ex2r[:], msqr[:])
        # B4b: one Ln + one Exp for all tiles (single table switch each)
        lnr_all = spool.tile([128, NRT], dt)
        nc.scalar.activation(lnr_all[:], varr_all[:], AF.Ln, bias=epsc[:])
        rstd_all = spool.tile([128, NRT], dt)
        i_rstd = nc.scalar.activation(rstd_all[:], lnr_all[:], AF.Exp,
                                      scale=-0.5)
        last_b4_act = i_rstd
        m2_all = spool.tile([128, NRT], dt)
        nc.vector.scalar_tensor_tensor(m2_all[:], mu_all[:], -1.0, rstd_all[:],
                                       op0=OP.mult, op1=OP.mult)
        # B4c: normalize + ln scale/shift
        for rt in range(NRT):
            a_n = an_all[:, rt, :]
            q = work2.tile([128, DM], dt, tag="q")
            nc.vector.tensor_scalar(q[:], a_n, rstd_all[:, rt:rt + 1],
                                    m2_all[:, rt:rt + 1],
                                    op0=OP.mult, op1=OP.add)
            ln = work2.tile([128, DM], dt, tag="ln")
            nc.vector.tensor_mul(ln[:], q[:], lnw_bc[:])
            nc.vector.tensor_add(an_all[:, rt, :], ln[:], lnb_bc[:])

        # B5: gelu + residual + transpose -> hbT (gelu table)
        last_gelu = None
        for rt in range(NRT):
            cs = rt * 128
            g = work2.tile([128, DM], dt, tag="g")
            i_g = nc.scalar.activation(g[:], an_all[:, rt, :], AF.Gelu)
            if rt == 0:
                add_dep_helper(i_g.ins, last_b4_act.ins, sync=True,
                               reason="act table: gelu after nle")
            last_gelu = i_g
            hb_row = work2.tile([128, DM], dt, tag="hb_row")
            nc.vector.tensor_add(hb_row[:], g[:], hrow_all[:, rt, :])
            ps_tb = mm_tile([DM, 128])
            nc.tensor.transpose(ps_tb[:], hb_row[:], ident[:])
            nc.scalar.copy(hbT[:, cs:cs + 128], ps_tb[:])

        if DEBUG:
            dma(dbg["d_hT"], hT[:])
            dma(dbg["d_hbT"], hbT[:])

        # ---- stage D: in_proj -> xcT (padded); z -> silu_z (padded) ----
        xcT = [big.tile([128, LT], dt, tag=f"xcT{cc}", name=f"xcT{cc}") for cc in range(2)]
        sluz = [big.tile([128, NROW], dtb, tag=f"sluz{cc}", name=f"sluz{cc}") for cc in range(2)]
        for cc in range(2):
            # only the gap columns need zeroing (conv taps read them)
            nc.gpsimd.memset(
                xcT[cc][:].rearrange("p (b l) -> p b l", b=RPC)[:, :, 0:GAP], 0.0)
        ccw = [(i * 512, min(512, NROW - i * 512))
               for i in range((NROW + 511) // 512)]
        first_silu = None
        for pc in range(4):
            cchunk, isx = (pc % 2), (pc < 2)
            for (c0, cw) in ccw:
                nblk_c = cw // PN
                ps_x = mm_tile([128, 512])
                nc.tensor.matmul(ps_x[:, :cw],
                                 w["in_proj_wT"][:, pc * 128:(pc + 1) * 128],
                                 hbT[:, c0:c0 + cw], start=True, stop=True)
                if isx:
                    p0 = (c0 // PN) * LP
                    dview = xcT[cchunk][:, p0:p0 + nblk_c * LP].rearrange(
                        "p (b l) -> p b l", b=nblk_c)[:, :, GAP:LP]
                    sview = ps_x[:, :cw].rearrange("p (b l) -> p b l", b=nblk_c)
                    nc.scalar.copy(dview, sview)
                else:
                    i_s = nc.scalar.activation(sluz[cchunk][:, c0:c0 + cw],
                                               ps_x[:, :cw], AF.Silu)
                    if first_silu is None:
                        first_silu = i_s
                        add_dep_helper(i_s.ins, last_gelu.ins, sync=True,
                                       reason="act table: silu after gelu")

        # ---- stage E: causal depthwise conv + silu (chunked, no in-place) ----
        xc2T = [big.tile([128, LT], dtb, tag=f"xc2T{cc}", name=f"xc2T{cc}")
                for cc in range(2)]
        for cc in range(2):
            nc.gpsimd.memset(
                xc2T[cc][:].rearrange("p (b l) -> p b l", b=RPC)[:, :, 0:GAP],
                0.0)
            wsl = w["conv_w2"][cc]
            for si in range(NSC):
                c0 = si * SCW
                cw_ = SCW - GAP
                t1c = scanp.tile([128, cw_], dtb, tag="a_t", name="cv1")
                nc.vector.tensor_scalar(t1c[:], xcT[cc][:, c0:c0 + cw_],
                                        wsl[:, 0:1], None, op0=OP.mult)
                t2c = scanp.tile([128, cw_], dtb, tag="b_t", name="cv2")
                nc.vector.scalar_tensor_tensor(t2c[:],
                                               xcT[cc][:, c0 + 1:c0 + 1 + cw_],
                                               wsl[:, 1:2], t1c[:],
                                               op0=OP.mult, op1=OP.add)
                t3c = scanp.tile([128, cw_], dtb, tag="a_t", name="t3c")
                nc.vector.scalar_tensor_tensor(t3c[:],
                                               xcT[cc][:, c0 + 2:c0 + 2 + cw_],
                                               wsl[:, 2:3], t2c[:],
                                               op0=OP.mult, op1=OP.add)
                t4c = scanp.tile([128, cw_], dtb, tag="b_t", name="t4c")
                nc.vector.scalar_tensor_tensor(t4c[:],
                                               xcT[cc][:, c0 + 3:c0 + 3 + cw_],
                                               wsl[:, 3:4], t3c[:],
                                               op0=OP.mult, op1=OP.add)
                i_cs = nc.scalar.activation(xc2T[cc][:, c0 + GAP:c0 + SCW],
                                            t4c[:], AF.Silu,
                                            bias=w["conv_b_col"][cc])
                last_silu = i_cs
        if DEBUG:
            for cc in range(2):
                dma(dbg["d_xc2T"][cc * 128:(cc + 1) * 128, :], xc2T[cc][:])

        # ---- stage F: x_proj -> dbl40 (dt rows 0:8, B 8:24, C 24:40) ----
        # one matmul + one PSUM->SBUF copy per scan chunk; dt rows start at
        # partition 0 so the dt_proj matmul base rule holds
        NDBL = DTR + 2 * DS
        dbl40_t = big.tile([NDBL, LT], dtb, tag="hT")  # reuse hT slot (dead)
        dbl40 = dbl40_t[:]
        dblD = dbl40[0:DTR, :]
        for si in range(NSC):
            c0 = si * SCW
            ps_d = mm_tile([NDBL, SCW])
            for h0, hw_ in ((0, 512), (512, SCW - 512)):
                for cc in range(2):
                    nc.tensor.matmul(ps_d[:, h0:h0 + hw_],
                                     w["x_proj_wT"][cc],
                                     xc2T[cc][:, c0 + h0:c0 + h0 + hw_],
                                     start=(cc == 0), stop=(cc == 1))
            nc.scalar.copy(dbl40[:, c0:c0 + SCW], ps_d[:])
        if DEBUG:
            dma(dbg["d_dblT"][0:DTR, :], dblD)
            dma(dbg["d_dblB"], dbl40[DTR:DTR + DS, :])
            dma(dbg["d_dblC"], dbl40[DTR + DS:NDBL, :])

        deltaT = [big.tile([128, LT], dtb, tag=f"xcT{cc}", name=f"deltaT{cc}") for cc in range(2)]
        duT = [big.tile([128, LT], dtb, tag=t, name=f"duT_{t}") for t in ("convacc", "hbT")]
        # Exp pass (staged into duT), then Ln pass -> softplus, grouped so the
        # act-table switches once per function
        for cc in range(2):
            for si in range(NSC):
                c0 = si * SCW
                ps_dt = mm_tile([128, SCW])
                for h0, hw_ in ((0, 512), (512, SCW - 512)):
                    nc.tensor.matmul(ps_dt[:, h0:h0 + hw_],
                                     w["dt_proj_wT"][:, cc * 128:(cc + 1) * 128],
                                     dbl40[0:DTR, c0 + h0:c0 + h0 + hw_],
                                     start=True, stop=True)
                i_e1 = nc.scalar.activation(duT[cc][:, c0:c0 + SCW], ps_dt[:],
                                            AF.Exp,
                                            bias=w["dt_proj_b_col"][cc])
                if cc == 0 and si == 0:
                    add_dep_helper(i_e1.ins, last_silu.ins, sync=True,
                                   reason="act table: exp after silu")
                last_exp_f = i_e1
        first_agen = None
        last_softplus = None
        for cc in range(2):
            for si in range(NSC):
                c0 = si * SCW
                i_ln = nc.scalar.activation(deltaT[cc][:, c0:c0 + SCW],
                                            duT[cc][:, c0:c0 + SCW],
                                            AF.Ln, bias=1.0)
                add_dep_helper(i_ln.ins, last_exp_f.ins, sync=True,
                               reason="act table: ln after exp pass")
                last_softplus = i_ln
                dv = lambda t: t[:, c0:c0 + SCW].rearrange(
                    "p (b l) -> p b l", b=SCB)
                nc.gpsimd.memset(dv(duT[cc])[:, :, 0:GAP], 0.0)
                nc.vector.tensor_mul(dv(duT[cc])[:, :, GAP:LP],
                                     dv(deltaT[cc])[:, :, GAP:LP],
                                     dv(xc2T[cc])[:, :, GAP:LP])
                # poison delta gaps so exp(A*delta)=0 there (state reset)
                nc.vector.memset(dv(deltaT[cc])[:, :, 0:GAP], POISON)
        if DEBUG:
            for cc in range(2):
                dma(dbg["d_deltaT"][cc * 128:(cc + 1) * 128, :], deltaT[cc][:])
                dma(dbg["d_duT"][cc * 128:(cc + 1) * 128, :], duT[cc][:])

        # ---- stage G: selective scan ----
        # B/C rows broadcast to 128 partitions via stride-0 free-dim DMA so
        # every scan-stage multiply is all-bf16 all-SBUF (DVE 2x mode)
        y2T = [big.tile([128, NROW], dtb, tag=f"y2T{cc}", name=f"y2T{cc}") for cc in range(2)]

        for si in range(NSC):
            c0 = si * SCW
            ps_ys = [ps_y.tile([128, SCW], dt, tag=f"ps_y{cc}", name=f"ps_ys{cc}")
                     for cc in range(2)]
            for d in range(DS):
                bm_t = scanp.tile([128, SCW], dtb, tag="bm_t", name="bm_t")
                dma(bm_t[:], dbl40[DTR + d:DTR + d + 1, c0:c0 + SCW]
                    .unsqueeze(1).broadcast_to((1, 128, SCW)))
                cm_t = scanp.tile([128, SCW], dtb, tag="cm_t", name="cm_t")
                dma(cm_t[:], dbl40[DTR + DS + d:DTR + DS + d + 1, c0:c0 + SCW]
                    .unsqueeze(1).broadcast_to((1, 128, SCW)))
                for cc in range(2):
                    a_t = scanp.tile([128, SCW], mybir.dt.bfloat16, tag="sc_a",
                                     name="a_t")
                    i_ag = nc.scalar.activation(a_t[:],
                                                deltaT[cc][:, c0:c0 + SCW],
                                                AF.Exp,
                                                scale=A_sb[cc][:, d:d + 1])
                    if first_agen is None:
                        first_agen = i_ag
                        add_dep_helper(i_ag.ins, last_softplus.ins, sync=True,
                                       reason="act table: exp after ln")
                    b_t = scanp.tile([128, SCW], mybir.dt.bfloat16, tag="sc_b",
                                     name="b_t")
                    nc.vector.tensor_mul(b_t[:], duT[cc][:, c0:c0 + SCW],
                                         bm_t[:])
                    h_t = scanp.tile([128, SCW], mybir.dt.bfloat16, tag="h_t")
                    nc.vector.tensor_tensor_scan(
                        h_t[:], a_t[:], b_t[:], initial=0.0,
                        op0=OP.mult, op1=OP.add)
                    p_t = scanp.tile([128, SCW], mybir.dt.bfloat16, tag="p_t")
                    # odd d: C-multiply on the otherwise-idle GPSIMD engine
                    peng = nc.gpsimd if (d % 2 == 1) else nc.vector
                    peng.tensor_mul(p_t[:], h_t[:], cm_t[:])
                    for h0, hw_ in ((0, 512), (512, SCW - 512)):
                        nc.tensor.matmul(ps_ys[cc][:, h0:h0 + hw_],
                                         identb[:], p_t[:, h0:h0 + hw_],
                                         start=(d == 0), stop=(d == DS - 1))
            d0 = si * SCB * PN
            for cc in range(2):
                t1s = scanp.tile([128, SCW], dtb, tag="t1s")
                nc.vector.scalar_tensor_tensor(
                    t1s[:], xc2T[cc][:, c0:c0 + SCW],
                    w["D_col"][cc], ps_ys[cc][:],
                    op0=OP.mult, op1=OP.add)
                nc.vector.tensor_mul(
                    y2T[cc][:, d0:d0 + SCB * PN].rearrange(
                        "p (b l) -> p b l", b=SCB),
                    t1s[:].rearrange("p (b l) -> p b l", b=SCB)[:, :, GAP:LP],
                    sluz[cc][:, d0:d0 + SCB * PN].rearrange(
                        "p (b l) -> p b l", b=SCB))
        if DEBUG:
            for cc in range(2):
                dma(dbg["d_y2T"][cc * 128:(cc + 1) * 128, :], y2T[cc][:])

        # ---- stage H: out_proj (compact, bf16 out) ----
        moT = big.tile([DM, NROW], mybir.dt.bfloat16, tag="sluz0", name="moT")
        CW = SCB * PN
        for si in range(NSC):
            d0 = si * CW
            ps_mo = mm_tile([DM, CW])
            for h0, hw_ in ((0, 512), (512, CW - 512)):
                for cc in range(2):
                    nc.tensor.matmul(ps_mo[:, h0:h0 + hw_],
                                     w["out_proj_wT"][cc],
                                     y2T[cc][:, d0 + h0:d0 + h0 + hw_],
                                     start=(cc == 0), stop=(cc == 1))
            nc.scalar.copy(moT[:, d0:d0 + CW], ps_mo[:])
        if DEBUG:
            dma(dbg["d_moT"], moT[:])
        scanp_cm.__exit__(None, None, None)

        # ---- stage I: mlp2 (gelu) + mlp3 + denorm + output ----
        w2pool = P(name="w2p", bufs=1)
        w2sb = w2pool.tile([DM, PN * 2 * PRED], mybir.dt.bfloat16)
        dma(w2sb[:], IN["w2s"])
        w2v = w2sb[:].rearrange("p (n j) -> p n j", n=PN)
        ps_o2 = ps_y.tile([128, RPC], dt, tag="ps_y0")
        ps_o2b = ps_y.tile([2 * PRED - 128, RPC], dt, tag="ps_y1")
        mo_v = moT[:].rearrange("p (b l) -> p b l", b=RPC)
        for pn in range(PN):
            rhs = mo_v[:, :, pn:pn + 1]
            nc.tensor.matmul(ps_o2[:], w2v[:, pn, 0:128], rhs,
                             start=(pn == 0), stop=(pn == PN - 1))
            nc.tensor.matmul(ps_o2b[:], w2v[:, pn, 128:2 * PRED], rhs,
                             start=(pn == 0), stop=(pn == PN - 1))
        o2a = work.tile([128, RPC], dt, tag="o2a")
        nc.scalar.activation(o2a[:], ps_o2[:], AF.Gelu,
                             bias=w["mlp2_b_col"][0])
        o2b = work.tile([2 * PRED - 128, RPC], dt, tag="o2b")
        nc.scalar.activation(o2b[:], ps_o2b[:], AF.Gelu,
                             bias=w["mlp2_b_col"][1])
        ps_o3 = mm_tile([PRED, RPC])
        nc.tensor.matmul(ps_o3[:], w["mlp3_wT"][0], o2a[:],
                         start=True, stop=False)
        nc.tensor.matmul(ps_o3[:], w["mlp3_wT"][1], o2b[:],
                         start=False, stop=True)
        o3T = work.tile([PRED, RPC], dt, tag="o3T")
        nc.vector.tensor_copy(o3T[:], ps_o3[:])
        ps_o3t = mm_tile([RPC, PRED])
        nc.tensor.transpose(ps_o3t[:], o3T[:], ident[0:PRED, 0:PRED])

        den = work.tile([RPC, PRED], dt, tag="den")
        nc.vector.tensor_scalar(den[:], b3_bc, t_den[:], u_den[:],
                                op0=OP.mult, op1=OP.add)
        out_sb = work.tile([RPC, PRED], dt, tag="out_sb")
        nc.vector.scalar_tensor_tensor(out_sb[:], ps_o3t[:], t_den[:], den[:],
                                       op0=OP.mult, op1=OP.add)
        dma(out_d, out_sb[:])

    nc.compile()
    _cache[key] = nc
    return nc


def kernel(**inputs):
    nc = build_program()
    in_maps = prep_inputs(inputs)
    res = run_bass_kernel_spmd(nc, in_maps, list(range(NCORES)))
    return assemble(res.results)


if __name__ == "__main__":
    import reference as R
    inp = R.setup_inputs()
    out = kernel(**{k: np.asarray(v) for k, v in inp.items()})
    print("kernel out", out.shape, out.dtype, np.abs(out).max())



# revision 83
# speedup vs baseline: 11.5989x; 1.0744x over previous
"""Trainium2 Bass kernel for nn_Backbone1_62947040690721.

Data-parallel over the fused B*NV block axis: 336 independent per-series
problems, 42 per NeuronCore across 8 cores. All weights replicated and
host-packed into two tensors (one f32, one bf16) so the load phase is two
DMA instructions; A = -exp(A_log), broadcast bias rows, and the mv ones
column are precomputed on the host.

Layouts (per core, 42 blocks):
  - row layout: (block,patch) rows on partitions (128 rows = 2 blocks/tile)
  - T layout:   features on partitions, l = concat of blocks on the free axis,
                padded to 67 cols/block (3 zero "gap" cols + 64 data cols).
                The gaps give causal-conv zero padding and selective-scan
                state resets between blocks for free.

The selective scan runs as hardware tensor_tensor_scan instructions, one per
(state dim d, channel chunk), scanning 14 blocks' timelines per call; scan
chunks are block-aligned so every chunk self-resets at its leading gap cols.
B[d]/C[d] rows are broadcast to 128 partitions with stride-0 free-dim DMAs so
every scan-stage multiply is all-bf16 all-SBUF (DVE 2x mode); most C-side
multiplies run on the otherwise-idle GPSIMD engine. Wide fp32 matmuls
(attention logits, in_proj) use f32r operands rounded by their producing
copy for full-rate 4-byte PE throughput. Matmul moving widths are split at
the 512-col PSUM bank boundary.
"""

import sys

sys.path.insert(0, "/opt/trn_rl_repo")

import numpy as np

import concourse.bass as bass
import concourse.mybir as mybir
import concourse.tile as tile
from concourse import bacc
from concourse.bass_utils import run_bass_kernel_spmd

F32 = mybir.dt.float32
F32R = mybir.dt.float32r
AF = mybir.ActivationFunctionType
OP = mybir.AluOpType
AX = mybir.AxisListType

# model dims
B, T, NV = 16, 512, 21
PS, STRIDE, PRED = 16, 8, 96
DM, DS, DC = 128, 16, 4
DIN = 2 * DM          # 256
DTR = 8
S_EA = 512
PN = (T - PS) // STRIDE + 1 + 1  # 64
EPS = 1e-5

NCORES = 8
NBLK = B * NV          # 336
RPC = NBLK // NCORES   # 42 blocks per core
NROW = RPC * PN        # 2688 compact rows per core
NRT = NROW // 128      # 21 row tiles
GAP = 3                # zero-pad cols before each block
LP = PN + GAP          # 67 padded cols per block
LT = RPC * LP          # 2814 padded timeline length
SCB = 14               # blocks per scan chunk
SCW = SCB * LP         # 938 scan chunk width (>=256 keeps fp32r full rate)
NSC = RPC // SCB       # 3 scan chunks
POISON = 1.0e30

DEBUG = False          # set True (before first kernel build) for stage taps
REPEAT = 1             # build the body N times (for differential timing)

_cache = {}


def _r(x):
    return np.ascontiguousarray(np.asarray(x, dtype=np.float32))


# packed-weight layouts: (name, rows, cols); offsets assigned in order
PACKF_SPEC = [
    ("mlp1_wT", PS, DM),
    ("mk_wT", DM, S_EA),
    ("in_proj_wT", DM, 2 * DIN),
    ("conv_w2_0", 128, DC), ("conv_w2_1", 128, DC),
    ("conv_b_col_0", 128, 1), ("conv_b_col_1", 128, 1),
    ("dt_proj_b_col_0", 128, 1), ("dt_proj_b_col_1", 128, 1),
    ("D_col_0", 128, 1), ("D_col_1", 128, 1),
    ("A_0", 128, DS), ("A_1", 128, DS),
    ("mlp2_b_col_0", 128, 1), ("mlp2_b_col_1", 2 * PRED - 128, 1),
    ("mlp3_wT_0", 128, PRED), ("mlp3_wT_1", 2 * PRED - 128, PRED),
    ("b1_bc", 128, DM), ("lnw_bc", 128, DM), ("lnb_bc", 128, DM),
    ("w1s_bc", 128, DM),
    ("b3_bc", RPC, PRED),
]
PACKB_SPEC = [
    ("x_proj_wT_0", 128, DTR + 2 * DS), ("x_proj_wT_1", 128, DTR + 2 * DS),
    ("out_proj_wT_0", 128, DM), ("out_proj_wT_1", 128, DM),
    ("dt_proj_wT", DTR, DIN),
    ("mv_aug", 128, 4 * (DM + 1)),
]


def _pack_layout(spec):
    off, out = 0, {}
    for name, rows, cols in spec:
        out[name] = (rows, off, off + cols)
        off += cols
    return out, off


PACKF_LAYOUT, NPF = _pack_layout(PACKF_SPEC)
PACKB_LAYOUT, NPB = _pack_layout(PACKB_SPEC)


def prep_inputs(inputs):
    """Full inputs -> per-core input maps (pure data movement on host)."""
    x = _r(inputs["x"])
    xbn = np.ascontiguousarray(x.transpose(0, 2, 1).reshape(NBLK, T))
    xp = np.concatenate([xbn, np.repeat(xbn[:, -1:], STRIDE, axis=1)], axis=1)
    idx = np.arange(PN)[:, None] * STRIDE + np.arange(PS)[None, :]
    pat = xp[:, idx]                                     # (336, 64, 16)
    patT = np.ascontiguousarray(pat.transpose(2, 0, 1))  # (16, 336, 64)
    wv = np.tile(_r(inputs["revin_w"]), B).reshape(NBLK, 1)
    bv = np.tile(_r(inputs["revin_b"]), B).reshape(NBLK, 1)

    import ml_dtypes
    bf16 = ml_dtypes.bfloat16
    mlp2_wT = _r(inputs["mlp2_w"]).T       # (8192, 192)
    w2s = np.ascontiguousarray(
        mlp2_wT.reshape(PN, DM, 2 * PRED).transpose(1, 0, 2)
    ).astype(bf16)  # (128, 64, 192) bf16

    mlp1_wT = _r(inputs["mlp1_w"]).T                     # (16,128)
    A = -np.exp(_r(inputs["A_log"]))                     # (256,16)
    conv_w2 = _r(inputs["conv_w"])[:, 0, :]              # (256,4)
    conv_b = _r(inputs["conv_b"]).reshape(DIN, 1)
    dt_b = _r(inputs["dt_proj_b"]).reshape(DIN, 1)
    D_col = _r(inputs["D_ssm"]).reshape(DIN, 1)
    mlp2_b = _r(inputs["mlp2_b"]).reshape(2 * PRED, 1)
    mlp3_wT = _r(inputs["mlp3_w"]).T                     # (192,96)
    fvals = {
        "mlp1_wT": mlp1_wT,
        "mk_wT": _r(inputs["mk_w"]).T,
        "in_proj_wT": _r(inputs["in_proj_w"]).T,
        "conv_w2_0": conv_w2[:128], "conv_w2_1": conv_w2[128:],
        "conv_b_col_0": conv_b[:128], "conv_b_col_1": conv_b[128:],
        "dt_proj_b_col_0": dt_b[:128], "dt_proj_b_col_1": dt_b[128:],
        "D_col_0": D_col[:128], "D_col_1": D_col[128:],
        "A_0": A[:128], "A_1": A[128:],
        "mlp2_b_col_0": mlp2_b[:128], "mlp2_b_col_1": mlp2_b[128:],
        "mlp3_wT_0": mlp3_wT[:128], "mlp3_wT_1": mlp3_wT[128:],
        "b1_bc": np.broadcast_to(_r(inputs["mlp1_b"])[None, :], (128, DM)),
        "lnw_bc": np.broadcast_to(_r(inputs["ln_w"])[None, :], (128, DM)),
        "lnb_bc": np.broadcast_to(_r(inputs["ln_b"])[None, :], (128, DM)),
        "w1s_bc": np.broadcast_to(mlp1_wT.sum(0)[None, :], (128, DM)),
        "b3_bc": np.broadcast_to(_r(inputs["mlp3_b"])[None, :], (RPC, PRED)),
    }
    packf = np.zeros((128, NPF), np.float32)
    for name, (rows, c0, c1) in PACKF_LAYOUT.items():
        packf[:rows, c0:c1] = fvals[name]

    mv_wT = _r(inputs["mv_w"]).T                         # (512,128)
    mv_aug = np.ones((128, 4 * (DM + 1)), np.float32)
    for sc in range(4):
        mv_aug[:, sc * 129:sc * 129 + DM] = mv_wT[sc * 128:(sc + 1) * 128]
    x_proj_wT = _r(inputs["x_proj_w"]).T                 # (256,40)
    out_proj_wT = _r(inputs["out_proj_w"]).T             # (256,128)
    bvals = {
        "x_proj_wT_0": x_proj_wT[:128], "x_proj_wT_1": x_proj_wT[128:],
        "out_proj_wT_0": out_proj_wT[:128], "out_proj_wT_1": out_proj_wT[128:],
        "dt_proj_wT": _r(inputs["dt_proj_w"]).T,
        "mv_aug": mv_aug,
    }
    packb = np.zeros((128, NPB), bf16)
    for name, (rows, c0, c1) in PACKB_LAYOUT.items():
        packb[:rows, c0:c1] = bvals[name].astype(bf16)

    shared = {"packf": packf, "packb": packb, "w2s": w2s}
    in_maps = []
    for c in range(NCORES):
        lo, hi = c * RPC, (c + 1) * RPC
        m = dict(shared)
        m["xrow"] = np.ascontiguousarray(xbn[lo:hi])                  # (42,512)
        m["patT"] = np.ascontiguousarray(patT[:, lo:hi, :]).reshape(PS, NROW)
        m["wv"] = np.ascontiguousarray(wv[lo:hi])
        m["bv"] = np.ascontiguousarray(bv[lo:hi])
        in_maps.append(m)
    return in_maps


def assemble(results):
    outs = np.concatenate([r["out"] for r in results], axis=0)  # (336, 96)
    out = outs.reshape(B, NV, PRED).transpose(0, 2, 1)
    return np.ascontiguousarray(out.astype(np.float32))


# ---------------------------------------------------------------------------
# program builder
# ---------------------------------------------------------------------------

def _decl_inputs(nc):
    d = {}
    spec = {
        "xrow": (RPC, T), "patT": (PS, NROW), "wv": (RPC, 1), "bv": (RPC, 1),
        "packf": (128, NPF), "packb": (128, NPB),
        "w2s": (DM, PN, 2 * PRED),
    }
    bf16_ins = ("w2s", "packb")
    for name, shape in spec.items():
        dty = mybir.dt.bfloat16 if name in bf16_ins else F32
        d[name] = nc.dram_tensor(name, list(shape), dty,
                                 kind="ExternalInput").ap()
    return d


def build_program():
    key = ("nc", REPEAT, DEBUG)
    if key in _cache:
        return _cache[key]
    nc = bacc.Bacc("TRN2", target_bir_lowering=False, debug=False,
                   num_devices=NCORES)
    IN = _decl_inputs(nc)
    out_d = nc.dram_tensor("out", [RPC, PRED], F32, kind="ExternalOutput").ap()

    dbg = {}
    if DEBUG:
        for name, shape in [
            ("d_hT", (DM, NROW)), ("d_hbT", (DM, NROW)),
            ("d_xc2T", (DIN, LT)), ("d_deltaT", (DIN, LT)),
            ("d_duT", (DIN, LT)), ("d_y2T", (DIN, NROW)),
            ("d_moT", (DM, NROW)), ("d_dblT", (DTR + 2 * DS, LT)),
            ("d_dblB", (DS, LT)), ("d_dblC", (DS, LT)),
            ("d_stats", (RPC, 6)),
        ]:
            dty = (mybir.dt.bfloat16
                   if name in ("d_dblB", "d_dblC", "d_moT") else F32)
            dbg[name] = nc.dram_tensor(name, list(shape), dty,
                                       kind="ExternalOutput").ap()

    from contextlib import ExitStack
    from concourse.masks import make_identity
    from concourse.tile import add_dep_helper

    with tile.TileContext(nc) as tc:
      for _rep in range(REPEAT):
       with ExitStack() as ctx:
        P = lambda **kw: ctx.enter_context(tc.tile_pool(**kw))
        wpool = P(name="weights", bufs=1)
        cpool = P(name="consts", bufs=1)
        spool = P(name="statp", bufs=1)
        big = P(name="bigact", bufs=1)
        work = P(name="work", bufs=4)
        work2 = P(name="work2", bufs=4)
        scanp_cm = tc.tile_pool(name="scanp", bufs=3)
        scanp = scanp_cm.__enter__()
        # PSUM: mm(2 bufs) + y = 8 banks max
        ps_mm = P(name="ps_mm", bufs=2, space="PSUM")
        ps_y = P(name="ps_y", bufs=1, space="PSUM")

        dt = F32
        dtb = mybir.dt.bfloat16

        def dma(dst, src):
            nc.sync.dma_start(out=dst, in_=src)

        def mm_tile(shape, tag="mm"):
            return ps_mm.tile(list(shape), dt, tag=tag, name=tag)

        # ---- constants ----
        ident = cpool.tile([128, 128], dt)
        make_identity(nc, ident[:])
        identb = cpool.tile([128, 128], mybir.dt.bfloat16)
        make_identity(nc, identb[:])
        epsc = cpool.tile([128, 1], dt)
        nc.vector.memset(epsc[:], EPS)

        # input data first so stage A starts immediately
        xr = big.tile([RPC, T], dt, tag="xrow")
        dma(xr[:], IN["xrow"])
        patT_sb = big.tile([PS, NROW], dt, tag="patT")
        dma(patT_sb[:], IN["patT"])
        wv = spool.tile([RPC, 1], dt)
        dma(wv[:], IN["wv"])
        bv = spool.tile([RPC, 1], dt)
        dma(bv[:], IN["bv"])

        # packed weights: one DMA per dtype (HWDGE slots are ~625ns each)
        packf_t = wpool.tile([128, NPF], dt)
        dma(packf_t[:], IN["packf"])
        packb_t = wpool.tile([128, NPB], dtb)
        dma(packb_t[:], IN["packb"])

        def WF(name):
            rows, c0, c1 = PACKF_LAYOUT[name]
            return packf_t[0:rows, c0:c1]

        def WB(name):
            rows, c0, c1 = PACKB_LAYOUT[name]
            return packb_t[0:rows, c0:c1]

        w = {
            "mlp1_wT": WF("mlp1_wT"), "mk_wT": WF("mk_wT"),
            "in_proj_wT": WF("in_proj_wT"),
            "dt_proj_wT": WB("dt_proj_wT"),
            "conv_w2": [WF("conv_w2_0"), WF("conv_w2_1")],
            "conv_b_col": [WF("conv_b_col_0"), WF("conv_b_col_1")],
            "dt_proj_b_col": [WF("dt_proj_b_col_0"), WF("dt_proj_b_col_1")],
            "D_col": [WF("D_col_0"), WF("D_col_1")],
            "x_proj_wT": [WB("x_proj_wT_0"), WB("x_proj_wT_1")],
            "out_proj_wT": [WB("out_proj_wT_0"), WB("out_proj_wT_1")],
            "mlp2_b_col": [WF("mlp2_b_col_0"), WF("mlp2_b_col_1")],
            "mlp3_wT": [WF("mlp3_wT_0"), WF("mlp3_wT_1")],
        }
        mv_aug = WB("mv_aug")
        A_sb = [WF("A_0"), WF("A_1")]
        b1_bc = WF("b1_bc")
        lnw_bc = WF("lnw_bc")
        lnb_bc = WF("lnb_bc")
        w1s_bc = WF("w1s_bc")
        b3_bc = WF("b3_bc")

        # ---- stage A: RevIN stats ----
        sumx = spool.tile([RPC, 1], dt)
        nc.vector.reduce_sum(sumx[:], xr[:], axis=AX.X)
        mean = spool.tile([RPC, 1], dt)
        nc.vector.tensor_scalar_mul(mean[:], sumx[:], 1.0 / T)
        sq = work.tile([RPC, T], dt, tag="sq", bufs=1)
        sumx2 = spool.tile([RPC, 1], dt)
        nc.vector.tensor_mul(sq[:], xr[:], xr[:])
        nc.vector.reduce_sum(sumx2[:], sq[:], axis=AX.X)
        ex2 = spool.tile([RPC, 1], dt)
        nc.vector.tensor_scalar_mul(ex2[:], sumx2[:], 1.0 / T)
        msq = spool.tile([RPC, 1], dt)
        nc.vector.tensor_mul(msq[:], mean[:], mean[:])
        var = spool.tile([RPC, 1], dt)
        nc.vector.tensor_sub(var[:], ex2[:], msq[:])
        lnv = spool.tile([RPC, 1], dt)
        nc.scalar.activation(lnv[:], var[:], AF.Ln, bias=epsc[0:RPC, :])
        std = spool.tile([RPC, 1], dt)
        nc.scalar.activation(std[:], lnv[:], AF.Exp, scale=0.5)
        istd = spool.tile([RPC, 1], dt)
        nc.scalar.activation(istd[:], lnv[:], AF.Exp, scale=-0.5)

        s_n = spool.tile([RPC, 1], dt)
        nc.vector.tensor_mul(s_n[:], wv[:], istd[:])
        o_n0 = spool.tile([RPC, 1], dt)
        nc.vector.scalar_tensor_tensor(o_n0[:], mean[:], -1.0, s_n[:],
                                       op0=OP.mult, op1=OP.mult)
        o_n = spool.tile([RPC, 1], dt)
        nc.vector.tensor_add(o_n[:], o_n0[:], bv[:])

        svec = spool.tile([RPC, 2], dt)
        nc.vector.tensor_copy(svec[:, 0:1], s_n[:])
        nc.vector.tensor_copy(svec[:, 1:2], o_n[:])

        # denorm factors (needed only at stage I) after svec so B1 starts
        wq = spool.tile([RPC, 1], dt)
        nc.vector.tensor_scalar_add(wq[:], wv[:], EPS * EPS)
        rw = spool.tile([RPC, 1], dt)
        nc.vector.reciprocal(rw[:], wq[:])
        t_den = spool.tile([RPC, 1], dt)
        nc.vector.tensor_mul(t_den[:], std[:], rw[:])
        u_den0 = spool.tile([RPC, 1], dt)
        nc.vector.scalar_tensor_tensor(u_den0[:], bv[:], -1.0, t_den[:],
                                       op0=OP.mult, op1=OP.mult)
        u_den = spool.tile([RPC, 1], dt)
        nc.vector.tensor_add(u_den[:], u_den0[:], mean[:])
        if DEBUG:
            stats = spool.tile([RPC, 6], dt)
            for i, tl in enumerate([mean, std, s_n, o_n, t_den, u_den]):
                nc.vector.tensor_copy(stats[:, i:i + 1], tl[:])
            dma(dbg["d_stats"], stats[:])

        # ---- stage B: mlp1 + external attention + LN + gelu + residual ----
        # structured as function-grouped passes to avoid ACT table thrash
        hT = big.tile([DM, NROW], F32R, tag="hT")
        hbT = big.tile([DM, NROW], F32R, tag="hbT")
        # one-time f32 -> f32r weight conversions (rounded for PE full rate)
        mkr_t = wpool.tile([DM, S_EA], F32R, tag="mkr")
        nc.scalar.copy(mkr_t[:], w["mk_wT"])
        inpr_t = wpool.tile([DM, 2 * DIN], F32R, tag="inpr")
        nc.scalar.copy(inpr_t[:], w["in_proj_wT"])
        hrow_all = big.tile([128, NRT, DM], dt, tag="sluz0")
        an_all = big.tile([128, NRT, DM], dt, tag="sluz1")
        # exp(logits), bf16, two halves (sc 0-1 / 2-3), each [128, NRT, 256]
        exp4 = [big.tile([128, NRT, 2 * 128], dtb, tag=tg, name=f"exp4_{i}")
                for i, tg in enumerate(["xcT0", "xcT1"])]

        # B1: mlp1 + revin fold + transpose -> hT, hrow_all
        for rt in range(NRT):
            cs = rt * 128
            so_row = work.tile([128, 2], dt, tag="so_row")
            dma(so_row[:],
                svec[rt * 2:rt * 2 + 2, :].unsqueeze(1).broadcast_to((2, PN, 2)))
            ps_h = mm_tile([128, DM])
            nc.tensor.matmul(ps_h[:], patT_sb[:, cs:cs + 128], w["mlp1_wT"],
                             start=True, stop=True)
            t1 = work.tile([128, DM], dt, tag="t1")
            nc.vector.scalar_tensor_tensor(t1[:], w1s_bc[:], so_row[:, 1:2],
                                           b1_bc[:], op0=OP.mult, op1=OP.add)
            nc.vector.scalar_tensor_tensor(hrow_all[:, rt, :], ps_h[:],
                                           so_row[:, 0:1], t1[:],
                                           op0=OP.mult, op1=OP.add)
            ps_tr = mm_tile([DM, 128])
            nc.tensor.transpose(ps_tr[:], hrow_all[:, rt, :], ident[:])
            nc.vector.tensor_copy(hT[:, cs:cs + 128], ps_tr[:])

        # B2: logits + exp. Row tiles paired so the f32r matmul moving width
        # is 256 (full-rate 4-byte PE mode); exp batched per (pair, key-half).
        for rt0 in range(0, NRT, 2):
            cs = rt0 * 128
            npair = 2 if rt0 + 1 < NRT else 1
            mw = 128 * npair
            for half in range(2):
                ps_l = mm_tile([128, 512])
                for j in range(2):
                    sc = half * 2 + j
                    nc.tensor.matmul(ps_l[:, j * 256:j * 256 + mw],
                                     mkr_t[:, sc * 128:(sc + 1) * 128],
                                     hT[:, cs:cs + mw],
                                     start=True, stop=True)
                    nc.scalar.activation(
                        exp4[half][:, rt0:rt0 + npair,
                                   j * 128:(j + 1) * 128],
                        ps_l[:, j * 256:j * 256 + mw].rearrange(
                            "p (r c) -> p r c", r=npair),
                        AF.Exp)

        # B3: attnv (+sum column) + normalize
        for rt in range(NRT):
            ps_at = ps_y.tile([128, DM + 1], dt, tag="ps_y0", name="ps_at")
            for sc in range(4):
                nc.tensor.matmul(ps_at[:],
                                 exp4[sc // 2][:, rt,
                                               (sc % 2) * 128:(sc % 2 + 1) * 128],
                                 mv_aug[:, sc * 129:(sc + 1) * 129],
                                 start=(sc == 0), stop=(sc == 3))
            rec = work.tile([128, 1], dt, tag="rec")
            nc.vector.reciprocal(rec[:], ps_at[:, DM:DM + 1])
            nc.vector.tensor_scalar_mul(an_all[:, rt, :], ps_at[:, 0:DM], rec[:])

        # B4a: LN stats for all row tiles (Square is in every act table)
        mu_all = spool.tile([128, NRT], dt)
        varr_all = spool.tile([128, NRT], dt)
        for rt in range(NRT):
            a_n = an_all[:, rt, :]
            sm = work.tile([128, 1], dt, tag="sm")
            nc.vector.reduce_sum(sm[:], a_n, axis=AX.X)
            nc.vector.tensor_scalar_mul(mu_all[:, rt:rt + 1], sm[:], 1.0 / DM)
            sqs = work2.tile([128, DM], dt, tag="sqs")
            ssq = work.tile([128, 1], dt, tag="ssq")
            nc.vector.tensor_mul(sqs[:], a_n, a_n)
            nc.vector.reduce_sum(ssq[:], sqs[:], axis=AX.X)
            ex2r = work.tile([128, 1], dt, tag="ex2r")
            nc.vector.tensor_scalar_mul(ex2r[:], ssq[:], 1.0 / DM)
            msqr = work.tile([128, 1], dt, tag="msqr")
            nc.vector.tensor_mul(msqr[:], mu_all[:, rt:rt + 1],
                                 mu_all[:, rt:rt + 1])
            nc.vector.tensor_sub(varr_all[:, rt:rt + 1], ex2r[:], msqr[:])
        # B4b: one Ln + one Exp for all tiles (single table switch each)
        lnr_all = spool.tile([128, NRT], dt)
        nc.scalar.activation(lnr_all[:], varr_all[:], AF.Ln, bias=epsc[:])
        rstd_all = spool.tile([128, NRT], dt)
        i_rstd = nc.scalar.activation(rstd_all[:], lnr_all[:], AF.Exp,
                                      scale=-0.5)
        last_b4_act = i_rstd
        m2_all = spool.tile([128, NRT], dt)
        nc.vector.scalar_tensor_tensor(m2_all[:], mu_all[:], -1.0, rstd_all[:],
                                       op0=OP.mult, op1=OP.mult)
        # B4c: normalize + ln scale/shift
        for rt in range(NRT):
            a_n = an_all[:, rt, :]
            q = work2.tile([128, DM], dt, tag="q")
            nc.vector.tensor_scalar(q[:], a_n, rstd_all[:, rt:rt + 1],
                                    m2_all[:, rt:rt + 1],
                                    op0=OP.mult, op1=OP.add)
            ln = work2.tile([128, DM], dt, tag="ln")
            nc.vector.tensor_mul(ln[:], q[:], lnw_bc[:])
            nc.vector.tensor_add(an_all[:, rt, :], ln[:], lnb_bc[:])

        # B5: gelu + residual + transpose -> hbT (gelu table)
        last_gelu = None
        for rt in range(NRT):
            cs = rt * 128
            g = work2.tile([128, DM], dt, tag="g")
            i_g = nc.scalar.activation(g[:], an_all[:, rt, :], AF.Gelu)
            if rt == 0:
                add_dep_helper(i_g.ins, last_b4_act.ins, sync=True,
                               reason="act table: gelu after nle")
            last_gelu = i_g
            hb_row = work2.tile([128, DM], dt, tag="hb_row")
            nc.vector.tensor_add(hb_row[:], g[:], hrow_all[:, rt, :])
            ps_tb = mm_tile([DM, 128])
            nc.tensor.transpose(ps_tb[:], hb_row[:], ident[:])
            nc.vector.tensor_copy(hbT[:, cs:cs + 128], ps_tb[:])

        if DEBUG:
            dma(dbg["d_hT"], hT[:])
            dma(dbg["d_hbT"], hbT[:])

        # ---- stage D: in_proj -> xcT (padded); z -> silu_z (padded) ----
        xcT = [big.tile([128, LT], dt, tag=f"xcT{cc}", name=f"xcT{cc}") for cc in range(2)]
        sluz = [big.tile([128, NROW], dtb, tag=f"sluz{cc}", name=f"sluz{cc}") for cc in range(2)]
        for cc in range(2):
            # only the gap columns need zeroing (conv taps read them)
            nc.gpsimd.memset(
                xcT[cc][:].rearrange("p (b l) -> p b l", b=RPC)[:, :, 0:GAP], 0.0)
        ccw = [(i * 512, min(512, NROW - i * 512))
               for i in range((NROW + 511) // 512)]
        first_silu = None
        for pc in range(4):
            cchunk, isx = (pc % 2), (pc < 2)
            for (c0, cw) in ccw:
                nblk_c = cw // PN
                ps_x = mm_tile([128, 512])
                # f32r: full-rate 4-byte matmul (moving width >= 256)
                nc.tensor.matmul(ps_x[:, :cw],
                                 inpr_t[:, pc * 128:(pc + 1) * 128],
                                 hbT[:, c0:c0 + cw], start=True, stop=True)
                if isx:
                    p0 = (c0 // PN) * LP
                    dview = xcT[cchunk][:, p0:p0 + nblk_c * LP].rearrange(
                        "p (b l) -> p b l", b=nblk_c)[:, :, GAP:LP]
                    sview = ps_x[:, :cw].rearrange("p (b l) -> p b l", b=nblk_c)
                    nc.scalar.copy(dview, sview)
                else:
                    i_s = nc.scalar.activation(sluz[cchunk][:, c0:c0 + cw],
                                               ps_x[:, :cw], AF.Silu)
                    if first_silu is None:
                        first_silu = i_s
                        add_dep_helper(i_s.ins, last_gelu.ins, sync=True,
                                       reason="act table: silu after gelu")

        # ---- stage E: causal depthwise conv + silu (chunked, no in-place) ----
        xc2T = [big.tile([128, LT], dtb, tag=f"xc2T{cc}", name=f"xc2T{cc}")
                for cc in range(2)]
        for cc in range(2):
            nc.gpsimd.memset(
                xc2T[cc][:].rearrange("p (b l) -> p b l", b=RPC)[:, :, 0:GAP],
                0.0)
            wsl = w["conv_w2"][cc]
            # separate tile tags per cc chain (keeps buffer rotations apart)
            ceng = nc.vector
            ctag = "g" if cc == 1 else ""
            for si in range(NSC):
                c0 = si * SCW
                cw_ = SCW - GAP
                t1c = scanp.tile([128, cw_], dtb, tag="a_t" + ctag,
                                 name="cv1", bufs=2)
                ceng.tensor_scalar(t1c[:], xcT[cc][:, c0:c0 + cw_],
                                   wsl[:, 0:1], None, op0=OP.mult)
                t2c = scanp.tile([128, cw_], dtb, tag="b_t" + ctag,
                                 name="cv2", bufs=2)
                ceng.scalar_tensor_tensor(t2c[:],
                                          xcT[cc][:, c0 + 1:c0 + 1 + cw_],
                                          wsl[:, 1:2], t1c[:],
                                          op0=OP.mult, op1=OP.add)
                t3c = scanp.tile([128, cw_], dtb, tag="a_t" + ctag,
                                 name="t3c", bufs=2)
                ceng.scalar_tensor_tensor(t3c[:],
                                          xcT[cc][:, c0 + 2:c0 + 2 + cw_],
                                          wsl[:, 2:3], t2c[:],
                                          op0=OP.mult, op1=OP.add)
                t4c = scanp.tile([128, cw_], dtb, tag="b_t" + ctag,
                                 name="t4c", bufs=2)
                ceng.scalar_tensor_tensor(t4c[:],
                                          xcT[cc][:, c0 + 3:c0 + 3 + cw_],
                                          wsl[:, 3:4], t3c[:],
                                          op0=OP.mult, op1=OP.add)
                i_cs = nc.scalar.activation(xc2T[cc][:, c0 + GAP:c0 + SCW],
                                            t4c[:], AF.Silu,
                                            bias=w["conv_b_col"][cc])
                last_silu = i_cs
        if DEBUG:
            for cc in range(2):
                dma(dbg["d_xc2T"][cc * 128:(cc + 1) * 128, :], xc2T[cc][:])

        # ---- stage F: x_proj -> dbl40 (dt rows 0:8, B 8:24, C 24:40) ----
        # one matmul + one PSUM->SBUF copy per scan chunk; dt rows start at
        # partition 0 so the dt_proj matmul base rule holds
        NDBL = DTR + 2 * DS
        dbl40_t = big.tile([NDBL, LT], dtb, tag="hT")  # reuse hT slot (dead)
        dbl40 = dbl40_t[:]
        dblD = dbl40[0:DTR, :]
        for si in range(NSC):
            c0 = si * SCW
            ps_d = mm_tile([NDBL, SCW])
            for h0, hw_ in ((0, 512), (512, SCW - 512)):
                for cc in range(2):
                    nc.tensor.matmul(ps_d[:, h0:h0 + hw_],
                                     w["x_proj_wT"][cc],
                                     xc2T[cc][:, c0 + h0:c0 + h0 + hw_],
                                     start=(cc == 0), stop=(cc == 1))
            nc.scalar.copy(dbl40[:, c0:c0 + SCW], ps_d[:])
        if DEBUG:
            dma(dbg["d_dblT"][0:DTR, :], dblD)
            dma(dbg["d_dblB"], dbl40[DTR:DTR + DS, :])
            dma(dbg["d_dblC"], dbl40[DTR + DS:NDBL, :])

        deltaT = [big.tile([128, LT], dtb, tag=f"xcT{cc}", name=f"deltaT{cc}") for cc in range(2)]
        duT = [big.tile([128, LT], dtb, tag=t, name=f"duT_{t}") for t in ("convacc", "hbT")]
        # Exp pass (staged into duT), then Ln pass -> softplus, grouped so the
        # act-table switches once per function
        for cc in range(2):
            for si in range(NSC):
                c0 = si * SCW
                ps_dt = mm_tile([128, SCW])
                for h0, hw_ in ((0, 512), (512, SCW - 512)):
                    nc.tensor.matmul(ps_dt[:, h0:h0 + hw_],
                                     w["dt_proj_wT"][:, cc * 128:(cc + 1) * 128],
                                     dbl40[0:DTR, c0 + h0:c0 + h0 + hw_],
                                     start=True, stop=True)
                i_e1 = nc.scalar.activation(duT[cc][:, c0:c0 + SCW], ps_dt[:],
                                            AF.Exp,
                                            bias=w["dt_proj_b_col"][cc])
                if cc == 0 and si == 0:
                    add_dep_helper(i_e1.ins, last_silu.ins, sync=True,
                                   reason="act table: exp after silu")
                last_exp_f = i_e1
        first_agen = None
        last_softplus = None
        for cc in range(2):
            for si in range(NSC):
                c0 = si * SCW
                i_ln = nc.scalar.activation(deltaT[cc][:, c0:c0 + SCW],
                                            duT[cc][:, c0:c0 + SCW],
                                            AF.Ln, bias=1.0)
                add_dep_helper(i_ln.ins, last_exp_f.ins, sync=True,
                               reason="act table: ln after exp pass")
                last_softplus = i_ln
                dv = lambda t: t[:, c0:c0 + SCW].rearrange(
                    "p (b l) -> p b l", b=SCB)
                nc.gpsimd.memset(dv(duT[cc])[:, :, 0:GAP], 0.0)
                nc.vector.tensor_mul(dv(duT[cc])[:, :, GAP:LP],
                                     dv(deltaT[cc])[:, :, GAP:LP],
                                     dv(xc2T[cc])[:, :, GAP:LP])
                # poison delta gaps so exp(A*delta)=0 there (state reset)
                nc.vector.memset(dv(deltaT[cc])[:, :, 0:GAP], POISON)
        if DEBUG:
            for cc in range(2):
                dma(dbg["d_deltaT"][cc * 128:(cc + 1) * 128, :], deltaT[cc][:])
                dma(dbg["d_duT"][cc * 128:(cc + 1) * 128, :], duT[cc][:])

        # ---- stage G: selective scan ----
        # B/C rows broadcast to 128 partitions via stride-0 free-dim DMA so
        # every scan-stage multiply is all-bf16 all-SBUF (DVE 2x mode)
        y2T = [big.tile([128, NROW], dtb, tag=f"y2T{cc}", name=f"y2T{cc}") for cc in range(2)]

        for si in range(NSC):
            c0 = si * SCW
            ps_ys = [ps_y.tile([128, SCW], dt, tag=f"ps_y{cc}", name=f"ps_ys{cc}")
                     for cc in range(2)]
            for d in range(DS):
                bm_t = scanp.tile([128, SCW], dtb, tag="bm_t", name="bm_t")
                dma(bm_t[:], dbl40[DTR + d:DTR + d + 1, c0:c0 + SCW]
                    .unsqueeze(1).broadcast_to((1, 128, SCW)))
                cm_t = scanp.tile([128, SCW], dtb, tag="cm_t", name="cm_t")
                dma(cm_t[:], dbl40[DTR + DS + d:DTR + DS + d + 1, c0:c0 + SCW]
                    .unsqueeze(1).broadcast_to((1, 128, SCW)))
                for cc in range(2):
                    a_t = scanp.tile([128, SCW], mybir.dt.bfloat16, tag="sc_a",
                                     name="a_t")
                    i_ag = nc.scalar.activation(a_t[:],
                                                deltaT[cc][:, c0:c0 + SCW],
                                                AF.Exp,
                                                scale=A_sb[cc][:, d:d + 1])
                    if first_agen is None:
                        first_agen = i_ag
                        add_dep_helper(i_ag.ins, last_softplus.ins, sync=True,
                                       reason="act table: exp after ln")
                    b_t = scanp.tile([128, SCW], mybir.dt.bfloat16, tag="sc_b",
                                     name="b_t")
                    nc.vector.tensor_mul(b_t[:], duT[cc][:, c0:c0 + SCW],
                                         bm_t[:])
                    h_t = scanp.tile([128, SCW], mybir.dt.bfloat16, tag="h_t")
                    nc.vector.tensor_tensor_scan(
                        h_t[:], a_t[:], b_t[:], initial=0.0,
                        op0=OP.mult, op1=OP.add)
                    p_t = scanp.tile([128, SCW], mybir.dt.bfloat16, tag="p_t")
                    # most d: C-multiply on the otherwise-idle GPSIMD engine
                    # (but keep the last d on DVE so the final ys-accumulate
                    # for the chunk isn't gated on Pool's slower throughput)
                    peng = (nc.gpsimd if (d % 4 != 0 and d != DS - 1)
                            else nc.vector)
                    peng.tensor_mul(p_t[:], h_t[:], cm_t[:])
                    for h0, hw_ in ((0, 512), (512, SCW - 512)):
                        nc.tensor.matmul(ps_ys[cc][:, h0:h0 + hw_],
                                         identb[:], p_t[:, h0:h0 + hw_],
                                         start=(d == 0), stop=(d == DS - 1))
            d0 = si * SCB * PN
            for cc in range(2):
                t1s = scanp.tile([128, SCW], dtb, tag="t1s")
                nc.vector.scalar_tensor_tensor(
                    t1s[:], xc2T[cc][:, c0:c0 + SCW],
                    w["D_col"][cc], ps_ys[cc][:],
                    op0=OP.mult, op1=OP.add)
                nc.gpsimd.tensor_mul(
                    y2T[cc][:, d0:d0 + SCB * PN].rearrange(
                        "p (b l) -> p b l", b=SCB),
                    t1s[:].rearrange("p (b l) -> p b l", b=SCB)[:, :, GAP:LP],
                    sluz[cc][:, d0:d0 + SCB * PN].rearrange(
                        "p (b l) -> p b l", b=SCB))
        if DEBUG:
            for cc in range(2):
                dma(dbg["d_y2T"][cc * 128:(cc + 1) * 128, :], y2T[cc][:])

        scanp_cm.__exit__(None, None, None)
        w2pool = P(name="w2p", bufs=1)
        w2sb = w2pool.tile([DM, PN * 2 * PRED], mybir.dt.bfloat16)
        dma(w2sb[:], IN["w2s"])

        # ---- stage H: out_proj (compact, bf16 out) ----
        moT = big.tile([DM, NROW], mybir.dt.bfloat16, tag="sluz0", name="moT")
        CW = SCB * PN
        for si in range(NSC):
            d0 = si * CW
            ps_mo = mm_tile([DM, CW])
            for h0, hw_ in ((0, 512), (512, CW - 512)):
                for cc in range(2):
                    nc.tensor.matmul(ps_mo[:, h0:h0 + hw_],
                                     w["out_proj_wT"][cc],
                                     y2T[cc][:, d0 + h0:d0 + h0 + hw_],
                                     start=(cc == 0), stop=(cc == 1))
            nc.scalar.copy(moT[:, d0:d0 + CW], ps_mo[:])
        if DEBUG:
            dma(dbg["d_moT"], moT[:])
        # ---- stage I: mlp2 (gelu) + mlp3 + denorm + output ----
        w2v = w2sb[:].rearrange("p (n j) -> p n j", n=PN)
        ps_o2 = ps_y.tile([128, RPC], dt, tag="ps_y0")
        ps_o2b = ps_y.tile([2 * PRED - 128, RPC], dt, tag="ps_y1")
        mo_v = moT[:].rearrange("p (b l) -> p b l", b=RPC)
        for pn in range(PN):
            rhs = mo_v[:, :, pn:pn + 1]
            nc.tensor.matmul(ps_o2[:], w2v[:, pn, 0:128], rhs,
                             start=(pn == 0), stop=(pn == PN - 1))
            nc.tensor.matmul(ps_o2b[:], w2v[:, pn, 128:2 * PRED], rhs,
                             start=(pn == 0), stop=(pn == PN - 1))
        o2a = work.tile([128, RPC], dt, tag="o2a")
        nc.scalar.activation(o2a[:], ps_o2[:], AF.Gelu,
                             bias=w["mlp2_b_col"][0])
        o2b = work.tile([2 * PRED - 128, RPC], dt, tag="o2b")
        nc.scalar.activation(o2b[:], ps_o2b[:], AF.Gelu,
                             bias=w["mlp2_b_col"][1])
        ps_o3 = mm_tile([PRED, RPC])
        nc.tensor.matmul(ps_o3[:], w["mlp3_wT"][0], o2a[:],
                         start=True, stop=False)
        nc.tensor.matmul(ps_o3[:], w["mlp3_wT"][1], o2b[:],
                         start=False, stop=True)
        o3T = work.tile([PRED, RPC], dt, tag="o3T")
        nc.vector.tensor_copy(o3T[:], ps_o3[:])
        ps_o3t = mm_tile([RPC, PRED])
        nc.tensor.transpose(ps_o3t[:], o3T[:], ident[0:PRED, 0:PRED])

        den = work.tile([RPC, PRED], dt, tag="den")
        nc.vector.tensor_scalar(den[:], b3_bc, t_den[:], u_den[:],
                                op0=OP.mult, op1=OP.add)
        out_sb = work.tile([RPC, PRED], dt, tag="out_sb")
        nc.vector.scalar_tensor_tensor(out_sb[:], ps_o3t[:], t_den[:], den[:],
                                       op0=OP.mult, op1=OP.add)
        dma(out_d, out_sb[:])

    nc.compile()
    _cache[key] = nc
    return nc


def kernel(**inputs):
    nc = build_program()
    in_maps = prep_inputs(inputs)
    res = run_bass_kernel_spmd(nc, in_maps, list(range(NCORES)))
    return assemble(res.results)


if __name__ == "__main__":
    import reference as R
    inp = R.setup_inputs()
    out = kernel(**{k: np.asarray(v) for k, v in inp.items()})
    print("kernel out", out.shape, out.dtype, np.abs(out).max())

